# revision 29
# baseline (speedup 1.0000x reference)
"""Trainium2 Bass kernel for causal multi-head attention with RoPE.

Full module: qkv = x @ w_qkv; RoPE(q, k); causal softmax attention;
out = attn_out @ w_out.  x: [2, 2048, 1024], 16 heads x 64 dim.

Sharding: 8 cores = 2 batches x 4 head-groups (4 heads/core).  Each core
computes its batch's q/k/v for its heads, runs attention, and produces a
partial [2048, 1024] output through its slice of w_out.  Host sums the 4
partials per batch.

v3.1 (on top of v2's st-major software pipeline + bf16 operands):
  - denominator folded into the AV matmul: V stationary is [128, 65]
    with a ones column, so the softmax denominator rides along as psum
    partition 64 (kills the 160 dedicated denominator matmuls, ~29us PE)
  - per-head psum banks av0..av3 [65, qt]; scores keep the [128, 2, qt]
    double-head layout but single-slot (tag scAB, 2 banks); qkv/pso
    double-buffer through wb/qv1
  - RoPE rotate_half via a PE permutation matmul (cross-partition moves
    on DVE/gpsimd are slow in the up direction; the PE does them at
    matmul speed).  The perm matmul for group i is emitted after group
    i+1's qkv matmuls so the PE never waits on the cast.
  - causal diag mask via gpsimd affine_select (no tri mask tensor)
  - softmax recip: den rows gathered by aligned down-copies into rrA
    rows {0,32,64} (heads 0..2) + rrB row 0 (head 3), exp(-ln(d)) on
    ACT, per-head broadcast matmuls into reused av banks
  - diagonal score matmuls restricted to [lo:qt] free range
  - input DMAs issued from three engines in parallel at startup;
    output stores merged to [128, 1024]
"""

import os
import sys

import numpy as np

for _p in ("/opt/trn_rl_repo", "/root/.axon_site/_ro/trn_rl_repo"):
    if os.path.isdir(_p) and _p not in sys.path:
        sys.path.append(_p)

import concourse.bass as bass
import concourse.mybir as mybir
import concourse.tile as tile

F32 = mybir.dt.float32
BF16 = mybir.dt.bfloat16

# Problem constants (hardcoded per contest rules)
B = 2
N = 2048
D = 1024
HEADS = 16
DH = 64
N_CORES = 8
HL = HEADS // (N_CORES // B)  # heads per core = 4

QT = 512
NQT = N // QT        # 4 query tiles
KC = D // 128        # 8 contraction chunks for qkv
CT = (HL * DH) // 128  # 2 column tiles for q/k (2 heads per tile)
NSB = N // 128       # 16 seq blocks
OC = CT              # w_out contraction chunks from this core
KPQ = QT // 128      # key chunks per query tile


def build_attention_nc(qt=QT, lag=5, cap_waits=True):
    n, d, hl, dh = N, D, HL, DH
    nhp = hl // 2
    scale = float(dh) ** -0.5
    nc = bass.Bass("TRN2", target_bir_lowering=False, debug=False)

    xT = nc.dram_tensor("xT", [NQT * d, qt], BF16, kind="ExternalInput").ap()
    wq = nc.dram_tensor("wq", [d, hl * dh], BF16, kind="ExternalInput").ap()
    wk = nc.dram_tensor("wk", [d, hl * dh], BF16, kind="ExternalInput").ap()
    wv = nc.dram_tensor("wv", [d, hl * dh], BF16, kind="ExternalInput").ap()
    wo = nc.dram_tensor("wo", [hl * dh, d], BF16, kind="ExternalInput").ap()
    cosT = nc.dram_tensor("cosT", [128, n], BF16, kind="ExternalInput").ap()
    sinT = nc.dram_tensor("sinT", [128, n], BF16, kind="ExternalInput").ap()
    selc = nc.dram_tensor("selc", [65, 192], BF16, kind="ExternalInput").ap()
    perm = nc.dram_tensor("perm", [128, 128], BF16, kind="ExternalInput").ap()
    outp = nc.dram_tensor("out_partial", [n, d], BF16, kind="ExternalOutput").ap()

    with tile.TileContext(nc) as tc:
        with (
            tc.tile_pool(name="pers", bufs=1) as pers,
            tc.tile_pool(name="ps", bufs=1, space="PSUM") as ps,
            tc.tile_pool(name="ropet", bufs=4) as ropet,
            tc.tile_pool(name="expp", bufs=12) as expp,
            tc.tile_pool(name="bcp", bufs=4) as bcp,
            tc.tile_pool(name="fo", bufs=4) as fo,
        ):
            # ---- persistent SBUF ----
            x_sb = [
                pers.tile([128, KC, qt], BF16, tag=f"x{s}", name=f"x{s}")
                for s in range(NQT)
            ]
            wq_sb = pers.tile([128, KC, hl * dh], BF16, tag="wq", name="wq")
            wk_sb = pers.tile([128, KC, hl * dh], BF16, tag="wk", name="wk")
            wv_sb = pers.tile([128, KC, hl * dh], BF16, tag="wv", name="wv")
            wo_sb = pers.tile([128, OC, d], BF16, tag="wo", name="wo")
            cos_sb = pers.tile([128, n], BF16, tag="cos", name="cos")
            sin_sb = pers.tile([128, n], BF16, tag="sin", name="sin")
            perm_sb = pers.tile([128, 128], BF16, tag="perm", name="perm")
            qT_sb = [pers.tile([128, n], BF16, tag=f"qT{i}", name=f"qT{i}") for i in range(CT)]
            kT_sb = [pers.tile([128, n], BF16, tag=f"kT{i}", name=f"kT{i}") for i in range(CT)]
            v_sb = [
                pers.tile([128, hl, dh + 1], BF16, tag=f"v{i}", name=f"v{i}")
                for i in range(NSB)
            ]
            outT_sb = [pers.tile([128, n], BF16, tag=f"oT{i}", name=f"oT{i}") for i in range(CT)]
            # unnormalized AV per (head, tile)
            u_sb = [
                [pers.tile([64, qt], F32, tag=f"u{h}_{t}", name=f"u{h}_{t}") for t in range(NQT)]
                for h in range(hl)
            ]
            # den gather rows: heads 0..2 at rrA rows {0,32,64}, head 3 at rrB
            rrA_f = pers.tile([65, qt], F32, tag="rrAf", name="rrAf")
            rrB_f = pers.tile([1, qt], F32, tag="rrBf", name="rrBf")
            # selector matrices (host-built): cols 0:128 broadcast rrA rows
            # {0,32} onto pbc01; cols 128:192 broadcast row 64 onto pbc23[0:64]
            selc_sb = pers.tile([65, 192], BF16, tag="selc", name="selc")
            ones1_sb = pers.tile([1, 64], BF16, tag="ones1", name="ones1")

            # ---- setup: v ones columns + rrA neutral fill + sel ones ----
            for sb in range(NSB):
                nc.vector.memset(v_sb[sb][:, :, dh : dh + 1], 1.0)
            nc.vector.memset(rrA_f, 1.0)
            nc.vector.memset(ones1_sb, 1.0)
            nc.gpsimd.dma_start(selc_sb, selc)
            nc.gpsimd.dma_start(perm_sb, perm)

            # ---- input DMA (bf16, host pre-cast), multi-engine issue.
            # sync: the first-matmul critical path (wq halves + x0 quarters);
            # gpsimd: wk, wv, x1, x3; scalar: cos, sin, x2, wo.
            hd = d // 2
            qd = d // 4
            nc.sync.dma_start(
                wq_sb[:, 0 : KC // 2, :],
                wq[0:hd, :].rearrange("(kc p) m -> p kc m", p=128),
            )
            for quarter in range(2):
                sl = slice(quarter * qd, (quarter + 1) * qd)
                nc.sync.dma_start(
                    x_sb[0][:, quarter * KC // 4 : (quarter + 1) * KC // 4, :],
                    xT[sl, :].rearrange("(kc p) m -> p kc m", p=128),
                )
            nc.sync.dma_start(
                wq_sb[:, KC // 2 : KC, :],
                wq[hd:d, :].rearrange("(kc p) m -> p kc m", p=128),
            )
            for quarter in range(2, 4):
                sl = slice(quarter * qd, (quarter + 1) * qd)
                nc.sync.dma_start(
                    x_sb[0][:, quarter * KC // 4 : (quarter + 1) * KC // 4, :],
                    xT[sl, :].rearrange("(kc p) m -> p kc m", p=128),
                )
            nc.gpsimd.dma_start(wk_sb, wk.rearrange("(kc p) m -> p kc m", p=128))
            nc.scalar.dma_start(cos_sb, cosT)
            nc.scalar.dma_start(sin_sb, sinT)
            nc.gpsimd.dma_start(wv_sb, wv.rearrange("(kc p) m -> p kc m", p=128))
            for s in range(1, NQT):
                eng = nc.gpsimd if s % 2 == 1 else nc.scalar
                eng.dma_start(
                    x_sb[s],
                    xT[s * d : (s + 1) * d, :].rearrange("(kc p) m -> p kc m", p=128),
                )
            nc.scalar.dma_start(wo_sb, wo.rearrange("(kc p) m -> p kc m", p=128))

            # Filler psum: single bank "wb" once attention starts (the score
            # slots need 4 banks).  During qkv(0) — before any scores — the
            # idle score slots serve as extra qkv buffers.
            qrot = {"i": 0, "pre": True}

            def alloc_fill(width, name):
                """Allocate a [128, width] psum region for qkv/pso work."""
                if qrot["pre"]:
                    j = qrot["i"] % 3
                    qrot["i"] += 1
                    if j > 0:
                        t = ps.tile([128, 2, qt], F32, tag=f"sc{'AB'[j - 1]}", name=name)
                        return t[:, 0, 0:width]
                t = ps.tile([128, width], F32, tag="wb", name=name)
                return t

            def qkv_groups(st):
                """Emit closures: qkv matmul groups with the RoPE tail of
                group i emitted after the matmuls of group i+1 (the perm
                matmul then never stalls the PE on the psum cast)."""
                qsl = slice(st * qt, (st + 1) * qt)
                ems = []

                def qk_mm(ct, qk, w_sb):
                    pq = alloc_fill(qt, f"pq{st}_{ct}_{qk}")
                    for kc in range(KC):
                        nc.tensor.matmul(
                            pq,
                            w_sb[:, kc, ct * 128 : (ct + 1) * 128],
                            x_sb[st][:, kc, :],
                            start=(kc == 0),
                            stop=(kc == KC - 1),
                        )
                    # raw (bf16, for the perm matmul) and the cos product are
                    # both read straight from psum; pq's lifetime stays inside
                    # this group so the wb/qv1 rotation remains safe
                    raw = ropet.tile([128, qt], BF16, tag="raw", name="raw")
                    nc.vector.tensor_copy(raw, pq)
                    a = ropet.tile([128, qt], BF16, tag="a", name="a")
                    nc.vector.tensor_tensor(a, pq, cos_sb[:, qsl], mybir.AluOpType.mult)
                    return raw, a

                def rope_tail(raw, a, ct, dst):
                    # rawS = perm @ raw on the PE (32-partition block swap);
                    # sin product reads the psum result directly
                    psP = alloc_fill(qt, "psP")
                    nc.tensor.matmul(psP, perm_sb, raw, start=True, stop=True)
                    sh = ropet.tile([128, qt], BF16, tag="sh", name="sh")
                    nc.vector.tensor_tensor(sh, psP, sin_sb[:, qsl], mybir.AluOpType.mult)
                    nc.gpsimd.tensor_tensor(dst[ct][:, qsl], a, sh, mybir.AluOpType.add)

                def v_group(j):
                    sb = st * KPQ + j
                    psv = alloc_fill(hl * dh, f"psv{sb}")
                    for kc in range(KC):
                        nc.tensor.matmul(
                            psv,
                            x_sb[st][:, kc, j * 128 : (j + 1) * 128],
                            wv_sb[:, kc, :],
                            start=(kc == 0),
                            stop=(kc == KC - 1),
                        )
                    nc.vector.tensor_copy(
                        v_sb[sb][:, :, 0:dh], psv.rearrange("p (h e) -> p h e", h=hl)
                    )

                # interleave: mm(i), mm(i+1), tail(i), mm(i+2), tail(i+1), ...
                pend = []  # (raw, a, ct, dst) awaiting tail

                def mk_mm(ct, qk, w_sb, dst):
                    def em():
                        raw, a = qk_mm(ct, qk, w_sb)
                        pend.append((raw, a, ct, dst))

                    return em

                def mk_tail():
                    def em():
                        raw, a, ct, dst = pend.pop(0)
                        rope_tail(raw, a, ct, dst)

                    return em

                qks = []
                for ct in range(CT):
                    for qk, w_sb, dst in ((0, wq_sb, qT_sb), (1, wk_sb, kT_sb)):
                        qks.append(mk_mm(ct, qk, w_sb, dst))
                ems.append(qks[0])
                ems.append(qks[1])
                ems.append(mk_tail())
                ems.append(qks[2])
                ems.append(mk_tail())
                ems.append(qks[3])
                ems.append(mk_tail())
                ems.append(lambda: v_group(0))
                ems.append(mk_tail())
                for j in range(1, KPQ):
                    ems.append(lambda j=j: v_group(j))
                return ems

            def emit_qkv(st):
                for em in qkv_groups(st):
                    em()

            attn_state = {}

            def emit_attn_main(t, fillers=(), start=0, reserve=0, inject=None):
                fillers = list(fillers)
                spread = fillers[: len(fillers) - reserve]
                reserved = fillers[len(fillers) - reserve :]
                qrot["pre"] = False
                qsl = slice(t * qt, (t + 1) * qt)
                ncc = KPQ * (t + 1)
                # AV/den psum banks: av01 pair-packs heads 0,1 [128, qt];
                # av2/av3 hold heads 2,3 as [0:64) AV + row 64 den, plus the
                # dens of heads 0,1 parked at row 96 by dedicated den matmuls.
                # Allocated lazily (first AV use) so the previous tile's
                # epilogue pbc tiles can be injected into this tile's weave
                # without inverting the av-bank tag order.
                pav = {}
                e_ts = {}
                scnt = {"i": 0}

                def emit_scores(c, hp):
                    j = c - KPQ * t
                    lo = max(0, j * 128)
                    pss = ps.tile(
                        [128, 2, qt], F32, tag=f"sc{'AB'[scnt['i'] % 2]}",
                        name=f"pss{t}_{c}_{hp}",
                    )
                    scnt["i"] += 1
                    for g in range(2):
                        bp = 64 * g
                        nc.tensor.matmul(
                            pss[:, g, lo:qt],
                            kT_sb[hp][bp : bp + dh, c * 128 : (c + 1) * 128],
                            qT_sb[hp][bp : bp + dh, t * qt + lo : (t + 1) * qt],
                            start=True,
                            stop=True,
                        )
                    e_t = expp.tile([128, 2, qt], BF16, tag="e", name="e")
                    nc.scalar.activation(
                        e_t[:, :, lo:qt], pss[:, :, lo:qt],
                        mybir.ActivationFunctionType.Exp, scale=scale,
                    )
                    if j >= 0:
                        # causal mask on the diagonal block: keep where
                        # query offset >= key offset
                        nc.gpsimd.affine_select(
                            out=e_t[:, :, lo : lo + 128],
                            in_=e_t[:, :, lo : lo + 128],
                            compare_op=mybir.AluOpType.is_ge,
                            fill=0.0,
                            base=0,
                            channel_multiplier=-1,
                            pattern=[[0, 2], [1, 128]],
                        )
                    e_ts[(c, hp)] = e_t

                def get_pav():
                    if "01" not in pav:
                        pav["01"] = ps.tile([128, qt], F32, tag="av01", name=f"pav{t}_01")
                        pav["2"] = ps.tile([97, qt], F32, tag="av2", name=f"pav{t}_2")
                        pav["3"] = ps.tile([97, qt], F32, tag="av3", name=f"pav{t}_3")
                    return pav

                def emit_av(c, hp):
                    lo = max(0, (c - KPQ * t) * 128)
                    e_t = e_ts.pop((c, hp))
                    p = get_pav()
                    st_fl = (c == 0)
                    sp_fl = (c == ncc - 1)
                    if hp == 0:
                        for g in range(2):
                            nc.tensor.matmul(
                                p["01"][64 * g : 64 * g + 64, lo:qt],
                                v_sb[c][:, g, 0:dh],
                                e_t[:, g, lo:qt],
                                start=st_fl, stop=sp_fl,
                                tile_position=(0, 64 * g),
                            )
                        # dens of heads 0,1 ride at row 96 of av2/av3
                        for g in range(2):
                            nc.tensor.matmul(
                                p[str(2 + g)][96:97, lo:qt],
                                v_sb[c][:, g, dh : dh + 1],
                                e_t[:, g, lo:qt],
                                start=st_fl, stop=sp_fl,
                                tile_position=(0, 96),
                            )
                    else:
                        for g in range(2):
                            nc.tensor.matmul(
                                p[str(2 + g)][0:65, lo:qt],
                                v_sb[c][:, 2 + g, :],
                                e_t[:, g, lo:qt],
                                start=st_fl, stop=sp_fl,
                            )

                done = 0
                nspread = len(spread)

                def advance(frac):
                    # weave filler groups: the PE keeps dense work that
                    # doesn't depend on ACT's exp stream
                    nonlocal done
                    if nspread:
                        want = min(nspread, int(frac * nspread))
                        while done < want:
                            spread[done]()
                            done += 1

                for c in range(ncc):
                    # interleave AV matmuls between the two serialized score
                    # pairs so the PE has work while ACT runs each exp
                    emit_scores(c, 0)
                    if c >= lag:
                        emit_av(c - lag, 0)
                    if c >= start:
                        advance((c - start + 0.5) / max(1, ncc - start))
                    emit_scores(c, 1)
                    if c >= lag:
                        emit_av(c - lag, 1)
                    if inject is not None and c == 1:
                        # previous tile's epilogue: its recip chain overlaps
                        # this tile's early chunks instead of blocking the PE
                        inject()
                        inject = None
                    if c >= start:
                        advance((c - start + 1.0) / max(1, ncc - start))
                for c in range(max(0, ncc - lag), ncc):
                    emit_av(c, 0)
                    emit_av(c, 1)
                advance(1.0)

                # softmax denominator path: aligned down-copies of the den
                # rows into rrA rows {0,32,64} (h=0..2) and rrB row 0 (h=3);
                # recip = exp(-ln(d)) on ACT
                p = get_pav()
                nc.vector.tensor_copy(rrA_f[0:1, :], p["2"][96:97, :])
                nc.vector.tensor_copy(rrA_f[32:33, :], p["3"][96:97, :])
                nc.vector.tensor_copy(rrA_f[64:65, :], p["2"][64:65, :])
                nc.vector.tensor_copy(rrB_f, p["3"][64:65, :])
                lnA = bcp.tile([65, qt], F32, tag="lnA", name="lnA")
                lnB = bcp.tile([1, qt], F32, tag="lnB", name="lnB")
                nc.scalar.activation(lnA, rrA_f, mybir.ActivationFunctionType.Ln)
                nc.scalar.activation(lnB, rrB_f, mybir.ActivationFunctionType.Ln)
                rrA = bcp.tile([65, qt], BF16, tag="rrA", name="rrA")
                rrB = bcp.tile([1, qt], BF16, tag="rrB", name="rrB")
                with nc.allow_low_precision(reason="bf16 softmax recip"):
                    nc.scalar.activation(
                        rrA, lnA, mybir.ActivationFunctionType.Exp, scale=-1.0
                    )
                    nc.scalar.activation(
                        rrB, lnB, mybir.ActivationFunctionType.Exp, scale=-1.0
                    )
                attn_state[t] = (rrA, rrB)

                # unnormalized AV -> SBUF, per head (aligned/down copies)
                nc.vector.tensor_copy(u_sb[0][t], p["01"][0:64, :])
                nc.vector.tensor_copy(u_sb[1][t], p["01"][64:128, :])
                nc.vector.tensor_copy(u_sb[2][t], p["2"][0:64, :])
                nc.vector.tensor_copy(u_sb[3][t], p["3"][0:64, :])

                # reserved fillers keep the PE busy while the recip chain runs
                for f in reserved:
                    f()

            def pso_closures(t):
                ems = []
                for j in range(KPQ):
                    sb = t * KPQ + j

                    def em(sb=sb):
                        o_t = fo.tile([128, 2, 512], BF16, tag="ot", name="ot")
                        for nt in range(2):
                            pso = alloc_fill(512, f"pso{sb}_{nt}")
                            for kc in range(OC):
                                nc.tensor.matmul(
                                    pso,
                                    outT_sb[kc][:, sb * 128 : (sb + 1) * 128],
                                    wo_sb[:, kc, nt * 512 : (nt + 1) * 512],
                                    start=(kc == 0),
                                    stop=(kc == OC - 1),
                                )
                            nc.vector.tensor_copy(o_t[:, nt, :], pso)
                        nc.sync.dma_start(
                            outp[sb * 128 : (sb + 1) * 128, :],
                            o_t.rearrange("p a b -> p (a b)"),
                        )

                    ems.append(em)
                return ems

            def emit_epi_head(t, col_split=False, interleave=None):
                qsl = slice(t * qt, (t + 1) * qt)
                rrA, rrB = attn_state.pop(t)
                # broadcast recips: pbc01 pair-packs heads 0,1; pbc23 heads
                # 2,3 (head 3 via a ones-vector matmul into partitions 64+)
                pbc01 = ps.tile([128, qt], F32, tag="av01", name=f"pbc{t}_01")
                nc.tensor.matmul(pbc01, selc_sb[:, 0:128], rrA, start=True, stop=True)
                pbc23 = ps.tile([128, qt], F32, tag="av2", name=f"pbc{t}_23")
                nc.tensor.matmul(
                    pbc23[0:64, :], selc_sb[:, 128:192], rrA, start=True, stop=True
                )
                nc.tensor.matmul(
                    pbc23[64:128, :], ones1_sb, rrB, start=True, stop=True,
                    tile_position=(0, 64),
                )
                pbc = [pbc01[0:64, :], pbc01[64:128, :], pbc23[0:64, :], pbc23[64:128, :]]
                if not col_split:
                    for h in range(hl):
                        nc.vector.tensor_tensor(
                            outT_sb[h // 2][64 * (h % 2) : 64 * (h % 2) + 64, qsl],
                            u_sb[h][t],
                            pbc[h],
                            mybir.AluOpType.mult,
                        )
                    return
                # column-split: pipeline the epi multiply with the trailing
                # out-projection so the kernel tail drains block by block
                interleave = list(interleave or [])
                for j in range(KPQ):
                    cs = slice(j * 128, (j + 1) * 128)
                    gs = slice(t * qt + j * 128, t * qt + (j + 1) * 128)
                    for h in range(hl):
                        nc.vector.tensor_tensor(
                            outT_sb[h // 2][64 * (h % 2) : 64 * (h % 2) + 64, gs],
                            u_sb[h][t][:, cs],
                            pbc[h][:, cs],
                            mybir.AluOpType.mult,
                        )
                    if j < len(interleave):
                        interleave[j]()

            # fillers skewed toward the later (longer, ACT-bound) tiles;
            # each tile's epilogue is injected into the next tile's weave
            emit_qkv(0)
            emit_attn_main(0, qkv_groups(1), reserve=2)
            emit_attn_main(1, qkv_groups(2), reserve=2,
                           inject=lambda: emit_epi_head(0))
            emit_attn_main(2, qkv_groups(3) + pso_closures(0), reserve=3,
                           inject=lambda: emit_epi_head(1))
            emit_attn_main(3, pso_closures(1) + pso_closures(2), start=1, reserve=5,
                           inject=lambda: emit_epi_head(2))
            emit_epi_head(3, col_split=True, interleave=pso_closures(3))

    if cap_waits:
        _cap_matmul_waits(nc)
    return nc


_CAPPED_INSTS = {
    "InstMatmult",
    "InstTensorTensor",
    "InstTensorCopy",
    "InstActivation",
    "InstTensorScalarAffineSelect",
    "InstTensorScalar",
    "InstTensorReduce",
    "InstMemset",
    "InstReciprocal",
    "InstLdweights",
    "InstTensorTensorScan",
    "InstIota",
    "InstDMACopy",
    "InstDrain",
}


def _cap_matmul_waits(nc, max_keep=1):
    """Walrus codegen allows only one sync-wait per compute instruction
    (S3 struct wait slots).  Move excess waits onto NoOps inserted just
    before, on the same engine; engines execute in order so the semantics
    are identical."""
    nop_id = 0
    for f in nc.m.functions:
        for blk in f.blocks:
            insts = blk.instructions
            idx = 0
            while idx < len(insts):
                inst = insts[idx]
                if (
                    type(inst).__name__ in _CAPPED_INSTS
                    and inst.sync_info is not None
                    and len(inst.sync_info.on_wait or []) > max_keep
                ):
                    waits = list(inst.sync_info.on_wait)
                    extra, keep = waits[:-max_keep], waits[-max_keep:]
                    inst.sync_info = mybir.SyncInfo(
                        on_wait=keep, on_update=list(inst.sync_info.on_update or [])
                    )
                    for w in extra:
                        nop = mybir.InstNoOp(name=f"I-mmwait-nop-{nop_id}")
                        nop_id += 1
                        nop.engine = inst.engine
                        nop.sync_info = mybir.SyncInfo(on_wait=[w], on_update=[])
                        insts.insert(idx, nop)
                        idx += 1
                idx += 1


def _rope_tables(n, dh):
    """Host-side RoPE tables in transposed, 2-head-stacked, sign-folded form."""
    inv_freq = 1.0 / (10000.0 ** (np.arange(0, dh, 2, dtype=np.float32) / dh))
    t = np.arange(n, dtype=np.float32)
    freqs = np.outer(t, inv_freq).astype(np.float32)  # [n, dh/2]
    emb = np.concatenate([freqs, freqs], axis=-1)  # [n, dh]
    cos = np.cos(emb).astype(np.float32).T  # [dh, n]
    sin = np.sin(emb).astype(np.float32).T
    sin_signed = sin.copy()
    sin_signed[: dh // 2] *= -1.0
    cosT = np.ascontiguousarray(np.tile(cos, (128 // dh, 1)))
    sinT = np.ascontiguousarray(np.tile(sin_signed, (128 // dh, 1)))
    return cosT, sinT


_NC_CACHE = {}


def kernel(x, w_qkv, w_out):
    return run(x, w_qkv, w_out)[0]


def _bf16(a):
    import ml_dtypes

    return np.asarray(a, dtype=np.float32).astype(ml_dtypes.bfloat16)


def run(x, w_qkv, w_out, trace=False, build_kwargs=None):
    from concourse.bass_utils import run_bass_kernel_spmd

    x = np.asarray(x, dtype=np.float32)
    w_qkv = np.asarray(w_qkv, dtype=np.float32)
    w_out = np.asarray(w_out, dtype=np.float32)

    cosT, sinT = _rope_tables(N, DH)
    # selector for the recip-broadcast matmuls: cols 0:128 map rrA rows
    # {0,32} to heads 0,1 of pbc01; cols 128:192 map row 64 to pbc23[0:64]
    # (head 3 uses a ones vector against rrB)
    selm = np.zeros((65, 192), dtype=np.float32)
    selm[0, 0:64] = 1.0
    selm[32, 64:128] = 1.0
    selm[64, 128:192] = 1.0
    # rotate_half permutation: permM[r, p] = 1 iff r = swap(p)
    permM = np.zeros((128, 128), dtype=np.float32)
    for p in range(128):
        sw = p + 32 if (p % 64) < 32 else p - 32
        permM[sw, p] = 1.0
    in_maps = []
    for core in range(N_CORES):
        b = core // (N_CORES // B)
        g = core % (N_CORES // B)
        cs = slice(g * HL * DH, (g + 1) * HL * DH)
        # x[b].T is [d, n]; reblock into NQT contiguous [d, qt] column blocks
        xt = np.ascontiguousarray(x[b].T)  # [d, n]
        xt_blocks = np.concatenate(
            [xt[:, s * QT : (s + 1) * QT] for s in range(NQT)], axis=0
        )  # [NQT*d, qt]
        in_maps.append(
            {
                "xT": _bf16(xt_blocks),
                "wq": _bf16(w_qkv[:, cs]),
                "wk": _bf16(w_qkv[:, D:][:, cs]),
                "wv": _bf16(w_qkv[:, 2 * D :][:, cs]),
                "wo": _bf16(w_out[cs, :]),
                "cosT": _bf16(cosT),
                "sinT": _bf16(sinT),
                "selc": _bf16(selm),
                "perm": _bf16(permM),
            }
        )

    key = repr(sorted((build_kwargs or {}).items()))
    if key not in _NC_CACHE:
        _NC_CACHE[key] = build_attention_nc(**(build_kwargs or {}))
    nc = _NC_CACHE[key]

    res = run_bass_kernel_spmd(
        nc, in_maps, core_ids=list(range(N_CORES)), trace=trace
    )
    out = np.zeros((B, N, D), dtype=np.float32)
    for core in range(N_CORES):
        out[core // (N_CORES // B)] += np.asarray(
            res.results[core]["out_partial"], dtype=np.float32
        )
    return out, res


if __name__ == "__main__":
    rng = np.random.default_rng(0)
    x = rng.standard_normal((B, N, D), dtype=np.float32)
    w_qkv = rng.standard_normal((D, 3 * D), dtype=np.float32) * D**-0.5
    w_out = rng.standard_normal((D, D), dtype=np.float32) * D**-0.5
    out = kernel(x, w_qkv, w_out)
    print("out", out.shape, out.dtype, float(np.abs(out).max()))


# revision 33
# speedup vs baseline: 1.0623x; 1.0623x over previous
"""Trainium2 Bass kernel for causal multi-head attention with RoPE.

Full module: qkv = x @ w_qkv; RoPE(q, k); causal softmax attention;
out = attn_out @ w_out.  x: [2, 2048, 1024], 16 heads x 64 dim.

Sharding: 8 cores = 2 batches x 4 head-groups (4 heads/core).  Each core
computes its batch's q/k/v for its heads, runs attention, and produces a
partial [2048, 1024] output through its slice of w_out.  Host sums the 4
partials per batch.

v3.1 (on top of v2's st-major software pipeline + bf16 operands):
  - denominator folded into the AV matmul: V stationary is [128, 65]
    with a ones column, so the softmax denominator rides along as psum
    partition 64 (kills the 160 dedicated denominator matmuls, ~29us PE)
  - per-head psum banks av0..av3 [65, qt]; scores keep the [128, 2, qt]
    double-head layout but single-slot (tag scAB, 2 banks); qkv/pso
    double-buffer through wb/qv1
  - RoPE rotate_half via a PE permutation matmul (cross-partition moves
    on DVE/gpsimd are slow in the up direction; the PE does them at
    matmul speed).  The perm matmul for group i is emitted after group
    i+1's qkv matmuls so the PE never waits on the cast.
  - causal diag mask via gpsimd affine_select (no tri mask tensor)
  - softmax recip: den rows gathered by aligned down-copies into rrA
    rows {0,32,64} (heads 0..2) + rrB row 0 (head 3), exp(-ln(d)) on
    ACT, per-head broadcast matmuls into reused av banks
  - diagonal score matmuls restricted to [lo:qt] free range
  - input DMAs issued from three engines in parallel at startup;
    output stores merged to [128, 1024]
"""

import os
import sys

import numpy as np

for _p in ("/opt/trn_rl_repo", "/root/.axon_site/_ro/trn_rl_repo"):
    if os.path.isdir(_p) and _p not in sys.path:
        sys.path.append(_p)

import concourse.bass as bass
import concourse.mybir as mybir
import concourse.tile as tile

F32 = mybir.dt.float32
BF16 = mybir.dt.bfloat16

# Problem constants (hardcoded per contest rules)
B = 2
N = 2048
D = 1024
HEADS = 16
DH = 64
N_CORES = 8
HL = HEADS // (N_CORES // B)  # heads per core = 4

QT = 512
NQT = N // QT        # 4 query tiles
KC = D // 128        # 8 contraction chunks for qkv
CT = (HL * DH) // 128  # 2 column tiles for q/k (2 heads per tile)
NSB = N // 128       # 16 seq blocks
OC = CT              # w_out contraction chunks from this core
KPQ = QT // 128      # key chunks per query tile


def build_attention_nc(qt=QT, lag=5, cap_waits=True):
    n, d, hl, dh = N, D, HL, DH
    nhp = hl // 2
    scale = float(dh) ** -0.5
    nc = bass.Bass("TRN2", target_bir_lowering=False, debug=False)

    xT = nc.dram_tensor("xT", [NQT * d, qt], BF16, kind="ExternalInput").ap()
    wq = nc.dram_tensor("wq", [d, hl * dh], BF16, kind="ExternalInput").ap()
    wk = nc.dram_tensor("wk", [d, hl * dh], BF16, kind="ExternalInput").ap()
    wv = nc.dram_tensor("wv", [d, hl * dh], BF16, kind="ExternalInput").ap()
    wo = nc.dram_tensor("wo", [hl * dh, d], BF16, kind="ExternalInput").ap()
    cosT = nc.dram_tensor("cosT", [128, n], BF16, kind="ExternalInput").ap()
    sinT = nc.dram_tensor("sinT", [128, n], BF16, kind="ExternalInput").ap()
    selc = nc.dram_tensor("selc", [65, 192], BF16, kind="ExternalInput").ap()
    perm = nc.dram_tensor("perm", [128, 128], BF16, kind="ExternalInput").ap()
    outp = nc.dram_tensor("out_partial", [n, d], BF16, kind="ExternalOutput").ap()

    with tile.TileContext(nc) as tc:
        with (
            tc.tile_pool(name="pers", bufs=1) as pers,
            tc.tile_pool(name="ps", bufs=1, space="PSUM") as ps,
            tc.tile_pool(name="ropet", bufs=4) as ropet,
            tc.tile_pool(name="expp", bufs=12) as expp,
            tc.tile_pool(name="bcp", bufs=4) as bcp,
            tc.tile_pool(name="fo", bufs=4) as fo,
        ):
            # ---- persistent SBUF ----
            x_sb = [
                pers.tile([128, KC, qt], BF16, tag=f"x{s}", name=f"x{s}")
                for s in range(NQT)
            ]
            wq_sb = pers.tile([128, KC, hl * dh], BF16, tag="wq", name="wq")
            wk_sb = pers.tile([128, KC, hl * dh], BF16, tag="wk", name="wk")
            wv_sb = pers.tile([128, KC, hl * dh], BF16, tag="wv", name="wv")
            wo_sb = pers.tile([128, OC, d], BF16, tag="wo", name="wo")
            cos_sb = pers.tile([128, n], BF16, tag="cos", name="cos")
            sin_sb = pers.tile([128, n], BF16, tag="sin", name="sin")
            perm_sb = pers.tile([128, 128], BF16, tag="perm", name="perm")
            qT_sb = [pers.tile([128, n], BF16, tag=f"qT{i}", name=f"qT{i}") for i in range(CT)]
            kT_sb = [pers.tile([128, n], BF16, tag=f"kT{i}", name=f"kT{i}") for i in range(CT)]
            v_sb = [
                pers.tile([128, hl, dh + 1], BF16, tag=f"v{i}", name=f"v{i}")
                for i in range(NSB)
            ]
            outT_sb = [pers.tile([128, n], BF16, tag=f"oT{i}", name=f"oT{i}") for i in range(CT)]
            # unnormalized AV per (head, tile)
            u_sb = [
                [pers.tile([64, qt], F32, tag=f"u{h}_{t}", name=f"u{h}_{t}") for t in range(NQT)]
                for h in range(hl)
            ]
            # den gather rows: heads 0..2 at rrA rows {0,32,64}, head 3 at rrB
            rrA_f = pers.tile([65, qt], F32, tag="rrAf", name="rrAf")
            rrB_f = pers.tile([1, qt], F32, tag="rrBf", name="rrBf")
            # selector matrices (host-built): cols 0:128 broadcast rrA rows
            # {0,32} onto pbc01; cols 128:192 broadcast row 64 onto pbc23[0:64]
            selc_sb = pers.tile([65, 192], BF16, tag="selc", name="selc")
            ones1_sb = pers.tile([1, 64], BF16, tag="ones1", name="ones1")

            # ---- setup: v ones columns + rrA neutral fill + sel ones ----
            for sb in range(NSB):
                nc.vector.memset(v_sb[sb][:, :, dh : dh + 1], 1.0)
            nc.vector.memset(rrA_f, 1.0)
            nc.vector.memset(ones1_sb, 1.0)
            nc.gpsimd.dma_start(selc_sb, selc)
            nc.gpsimd.dma_start(perm_sb, perm)

            # ---- input DMA (bf16, host pre-cast), multi-engine issue.
            # sync: the first-matmul critical path (wq halves + x0 quarters);
            # gpsimd: wk, wv, x1, x3; scalar: cos, sin, x2, wo.
            hd = d // 2
            qd = d // 4
            nc.sync.dma_start(
                wq_sb[:, 0 : KC // 2, :],
                wq[0:hd, :].rearrange("(kc p) m -> p kc m", p=128),
            )
            for quarter in range(2):
                sl = slice(quarter * qd, (quarter + 1) * qd)
                nc.sync.dma_start(
                    x_sb[0][:, quarter * KC // 4 : (quarter + 1) * KC // 4, :],
                    xT[sl, :].rearrange("(kc p) m -> p kc m", p=128),
                )
            nc.sync.dma_start(
                wq_sb[:, KC // 2 : KC, :],
                wq[hd:d, :].rearrange("(kc p) m -> p kc m", p=128),
            )
            for quarter in range(2, 4):
                sl = slice(quarter * qd, (quarter + 1) * qd)
                nc.sync.dma_start(
                    x_sb[0][:, quarter * KC // 4 : (quarter + 1) * KC // 4, :],
                    xT[sl, :].rearrange("(kc p) m -> p kc m", p=128),
                )
            nc.gpsimd.dma_start(wk_sb, wk.rearrange("(kc p) m -> p kc m", p=128))
            nc.scalar.dma_start(cos_sb, cosT)
            nc.scalar.dma_start(sin_sb, sinT)
            nc.gpsimd.dma_start(wv_sb, wv.rearrange("(kc p) m -> p kc m", p=128))
            for s in range(1, NQT):
                eng = nc.gpsimd if s % 2 == 1 else nc.scalar
                eng.dma_start(
                    x_sb[s],
                    xT[s * d : (s + 1) * d, :].rearrange("(kc p) m -> p kc m", p=128),
                )
            nc.scalar.dma_start(wo_sb, wo.rearrange("(kc p) m -> p kc m", p=128))

            # Filler psum: single bank "wb" once attention starts (the score
            # slots need 4 banks).  During qkv(0) — before any scores — the
            # idle score slots serve as extra qkv buffers.
            qrot = {"i": 0, "pre": True}

            def alloc_fill(width, name):
                """Allocate a [128, width] psum region for qkv/pso work."""
                if qrot["pre"]:
                    j = qrot["i"] % 3
                    qrot["i"] += 1
                    if j > 0:
                        t = ps.tile([128, 2, qt], F32, tag=f"sc{'AB'[j - 1]}", name=name)
                        return t[:, 0, 0:width]
                t = ps.tile([128, width], F32, tag="wb", name=name)
                return t

            def qkv_groups(st):
                """Emit closures: qkv matmul groups with the RoPE tail of
                group i emitted after the matmuls of group i+1 (the perm
                matmul then never stalls the PE on the psum cast)."""
                qsl = slice(st * qt, (st + 1) * qt)
                ems = []

                def qk_mm(ct, qk, w_sb):
                    pq = alloc_fill(qt, f"pq{st}_{ct}_{qk}")
                    for kc in range(KC):
                        nc.tensor.matmul(
                            pq,
                            w_sb[:, kc, ct * 128 : (ct + 1) * 128],
                            x_sb[st][:, kc, :],
                            start=(kc == 0),
                            stop=(kc == KC - 1),
                        )
                    # raw (bf16, for the perm matmul) and the cos product are
                    # both read straight from psum; pq's lifetime stays inside
                    # this group so the wb/qv1 rotation remains safe
                    raw = ropet.tile([128, qt], BF16, tag="raw", name="raw")
                    nc.vector.tensor_copy(raw, pq)
                    a = ropet.tile([128, qt], BF16, tag="a", name="a")
                    nc.vector.tensor_tensor(a, pq, cos_sb[:, qsl], mybir.AluOpType.mult)
                    return raw, a

                def rope_tail(raw, a, ct, dst):
                    # rawS = perm @ raw on the PE (32-partition block swap);
                    # sin product reads the psum result directly
                    psP = alloc_fill(qt, "psP")
                    nc.tensor.matmul(psP, perm_sb, raw, start=True, stop=True)
                    sh = ropet.tile([128, qt], BF16, tag="sh", name="sh")
                    nc.vector.tensor_tensor(sh, psP, sin_sb[:, qsl], mybir.AluOpType.mult)
                    nc.gpsimd.tensor_tensor(dst[ct][:, qsl], a, sh, mybir.AluOpType.add)

                def v_group(j):
                    sb = st * KPQ + j
                    psv = alloc_fill(hl * dh, f"psv{sb}")
                    for kc in range(KC):
                        nc.tensor.matmul(
                            psv,
                            x_sb[st][:, kc, j * 128 : (j + 1) * 128],
                            wv_sb[:, kc, :],
                            start=(kc == 0),
                            stop=(kc == KC - 1),
                        )
                    nc.vector.tensor_copy(
                        v_sb[sb][:, :, 0:dh], psv.rearrange("p (h e) -> p h e", h=hl)
                    )

                # interleave: mm(i), mm(i+1), tail(i), mm(i+2), tail(i+1), ...
                pend = []  # (raw, a, ct, dst) awaiting tail

                def mk_mm(ct, qk, w_sb, dst):
                    def em():
                        raw, a = qk_mm(ct, qk, w_sb)
                        pend.append((raw, a, ct, dst))

                    return em

                def mk_tail():
                    def em():
                        raw, a, ct, dst = pend.pop(0)
                        rope_tail(raw, a, ct, dst)

                    return em

                qks = []
                for ct in range(CT):
                    for qk, w_sb, dst in ((0, wq_sb, qT_sb), (1, wk_sb, kT_sb)):
                        qks.append(mk_mm(ct, qk, w_sb, dst))
                ems.append(qks[0])
                ems.append(qks[1])
                ems.append(mk_tail())
                ems.append(qks[2])
                ems.append(mk_tail())
                ems.append(qks[3])
                ems.append(mk_tail())
                ems.append(lambda: v_group(0))
                ems.append(mk_tail())
                for j in range(1, KPQ):
                    ems.append(lambda j=j: v_group(j))
                return ems

            def emit_qkv(st):
                for em in qkv_groups(st):
                    em()

            attn_state = {}

            def emit_attn_main(t, fillers=(), start=0, reserve=0, inject=None):
                fillers = list(fillers)
                spread = fillers[: len(fillers) - reserve]
                reserved = fillers[len(fillers) - reserve :]
                qrot["pre"] = False
                qsl = slice(t * qt, (t + 1) * qt)
                ncc = KPQ * (t + 1)
                # AV/den psum banks: av01 pair-packs heads 0,1 [128, qt];
                # av2/av3 hold heads 2,3 as [0:64) AV + row 64 den, plus the
                # dens of heads 0,1 parked at row 96 by dedicated den matmuls.
                # Allocated lazily (first AV use) so the previous tile's
                # epilogue pbc tiles can be injected into this tile's weave
                # without inverting the av-bank tag order.
                pav = {}
                e_ts = {}
                scnt = {"i": 0}

                def emit_scores(c, hp):
                    j = c - KPQ * t
                    lo = max(0, j * 128)
                    pss = ps.tile(
                        [128, 2, qt], F32, tag=f"sc{'AB'[scnt['i'] % 2]}",
                        name=f"pss{t}_{c}_{hp}",
                    )
                    scnt["i"] += 1
                    for g in range(2):
                        bp = 64 * g
                        nc.tensor.matmul(
                            pss[:, g, lo:qt],
                            kT_sb[hp][bp : bp + dh, c * 128 : (c + 1) * 128],
                            qT_sb[hp][bp : bp + dh, t * qt + lo : (t + 1) * qt],
                            start=True,
                            stop=True,
                        )
                    e_t = expp.tile([128, 2, qt], BF16, tag="e", name="e")
                    nc.scalar.activation(
                        e_t[:, :, lo:qt], pss[:, :, lo:qt],
                        mybir.ActivationFunctionType.Exp, scale=scale,
                    )
                    if j >= 0:
                        # causal mask on the diagonal block: keep where
                        # query offset >= key offset
                        nc.gpsimd.affine_select(
                            out=e_t[:, :, lo : lo + 128],
                            in_=e_t[:, :, lo : lo + 128],
                            compare_op=mybir.AluOpType.is_ge,
                            fill=0.0,
                            base=0,
                            channel_multiplier=-1,
                            pattern=[[0, 2], [1, 128]],
                        )
                    e_ts[(c, hp)] = e_t

                def get_pav():
                    if "01" not in pav:
                        pav["01"] = ps.tile([128, qt], F32, tag="av01", name=f"pav{t}_01")
                        pav["2"] = ps.tile([97, qt], F32, tag="av2", name=f"pav{t}_2")
                        pav["3"] = ps.tile([97, qt], F32, tag="av3", name=f"pav{t}_3")
                    return pav

                def emit_av(c, hp):
                    lo = max(0, (c - KPQ * t) * 128)
                    e_t = e_ts.pop((c, hp))
                    p = get_pav()
                    st_fl = (c == 0)
                    sp_fl = (c == ncc - 1)
                    if hp == 0:
                        for g in range(2):
                            nc.tensor.matmul(
                                p["01"][64 * g : 64 * g + 64, lo:qt],
                                v_sb[c][:, g, 0:dh],
                                e_t[:, g, lo:qt],
                                start=st_fl, stop=sp_fl,
                                tile_position=(0, 64 * g),
                            )
                        # dens of heads 0,1 ride at row 96 of av2/av3
                        for g in range(2):
                            nc.tensor.matmul(
                                p[str(2 + g)][96:97, lo:qt],
                                v_sb[c][:, g, dh : dh + 1],
                                e_t[:, g, lo:qt],
                                start=st_fl, stop=sp_fl,
                                tile_position=(0, 96),
                            )
                    else:
                        for g in range(2):
                            nc.tensor.matmul(
                                p[str(2 + g)][0:65, lo:qt],
                                v_sb[c][:, 2 + g, :],
                                e_t[:, g, lo:qt],
                                start=st_fl, stop=sp_fl,
                            )

                done = 0
                nspread = len(spread)

                def advance(frac):
                    # weave filler groups: the PE keeps dense work that
                    # doesn't depend on ACT's exp stream
                    nonlocal done
                    if nspread:
                        want = min(nspread, int(frac * nspread))
                        while done < want:
                            spread[done]()
                            done += 1

                for c in range(ncc):
                    # interleave AV matmuls between the two serialized score
                    # pairs so the PE has work while ACT runs each exp
                    emit_scores(c, 0)
                    if c >= lag:
                        emit_av(c - lag, 0)
                    if c >= start:
                        advance((c - start + 0.5) / max(1, ncc - start))
                    emit_scores(c, 1)
                    if c >= lag:
                        emit_av(c - lag, 1)
                    if inject is not None and c == 1:
                        # previous tile's epilogue: its recip chain overlaps
                        # this tile's early chunks instead of blocking the PE
                        inject()
                        inject = None
                    if c >= start:
                        advance((c - start + 1.0) / max(1, ncc - start))
                for c in range(max(0, ncc - lag), ncc):
                    emit_av(c, 0)
                    emit_av(c, 1)
                advance(1.0)

                # softmax denominator path: aligned down-copies of the den
                # rows into rrA rows {0,32,64} (h=0..2) and rrB row 0 (h=3);
                # recip = exp(-ln(d)) on ACT
                p = get_pav()
                # on the last tile ACT's exp stream is done: split the gather
                # and u copies across DVE and ACT to halve the tail chain
                alt = nc.scalar.copy if t == NQT - 1 else nc.vector.tensor_copy
                nc.vector.tensor_copy(rrA_f[0:1, :], p["2"][96:97, :])
                alt(rrA_f[32:33, :], p["3"][96:97, :])
                nc.vector.tensor_copy(rrA_f[64:65, :], p["2"][64:65, :])
                alt(rrB_f, p["3"][64:65, :])
                lnA = bcp.tile([65, qt], F32, tag="lnA", name="lnA")
                lnB = bcp.tile([1, qt], F32, tag="lnB", name="lnB")
                nc.scalar.activation(lnA, rrA_f, mybir.ActivationFunctionType.Ln)
                nc.scalar.activation(lnB, rrB_f, mybir.ActivationFunctionType.Ln)
                rrA = bcp.tile([65, qt], BF16, tag="rrA", name="rrA")
                rrB = bcp.tile([1, qt], BF16, tag="rrB", name="rrB")
                with nc.allow_low_precision(reason="bf16 softmax recip"):
                    nc.scalar.activation(
                        rrA, lnA, mybir.ActivationFunctionType.Exp, scale=-1.0
                    )
                    nc.scalar.activation(
                        rrB, lnB, mybir.ActivationFunctionType.Exp, scale=-1.0
                    )
                attn_state[t] = (rrA, rrB)

                # unnormalized AV -> SBUF, per head (aligned/down copies)
                nc.vector.tensor_copy(u_sb[0][t], p["01"][0:64, :])
                alt(u_sb[1][t], p["01"][64:128, :])
                nc.vector.tensor_copy(u_sb[2][t], p["2"][0:64, :])
                alt(u_sb[3][t], p["3"][0:64, :])

                # reserved fillers keep the PE busy while the recip chain runs
                for f in reserved:
                    f()

            def pso_closures(t):
                ems = []
                for j in range(KPQ):
                    sb = t * KPQ + j

                    def em(sb=sb):
                        o_t = fo.tile([128, 2, 512], BF16, tag="ot", name="ot")
                        for nt in range(2):
                            pso = alloc_fill(512, f"pso{sb}_{nt}")
                            for kc in range(OC):
                                nc.tensor.matmul(
                                    pso,
                                    outT_sb[kc][:, sb * 128 : (sb + 1) * 128],
                                    wo_sb[:, kc, nt * 512 : (nt + 1) * 512],
                                    start=(kc == 0),
                                    stop=(kc == OC - 1),
                                )
                            # in the drain tail (pre-mode), ACT is idle:
                            # alternate the psum->sbuf copies across engines
                            if qrot["pre"] and nt == 1:
                                nc.scalar.copy(o_t[:, nt, :], pso)
                            else:
                                nc.vector.tensor_copy(o_t[:, nt, :], pso)
                        nc.sync.dma_start(
                            outp[sb * 128 : (sb + 1) * 128, :],
                            o_t.rearrange("p a b -> p (a b)"),
                        )

                    ems.append(em)
                return ems

            def emit_epi_head(t, col_split=False, interleave=None):
                qsl = slice(t * qt, (t + 1) * qt)
                rrA, rrB = attn_state.pop(t)
                # broadcast recips: pbc01 pair-packs heads 0,1; pbc23 heads
                # 2,3 (head 3 via a ones-vector matmul into partitions 64+)
                pbc01 = ps.tile([128, qt], F32, tag="av01", name=f"pbc{t}_01")
                nc.tensor.matmul(pbc01, selc_sb[:, 0:128], rrA, start=True, stop=True)
                pbc23 = ps.tile([128, qt], F32, tag="av2", name=f"pbc{t}_23")
                nc.tensor.matmul(
                    pbc23[0:64, :], selc_sb[:, 128:192], rrA, start=True, stop=True
                )
                nc.tensor.matmul(
                    pbc23[64:128, :], ones1_sb, rrB, start=True, stop=True,
                    tile_position=(0, 64),
                )
                pbc = [pbc01[0:64, :], pbc01[64:128, :], pbc23[0:64, :], pbc23[64:128, :]]
                if not col_split:
                    for h in range(hl):
                        nc.vector.tensor_tensor(
                            outT_sb[h // 2][64 * (h % 2) : 64 * (h % 2) + 64, qsl],
                            u_sb[h][t],
                            pbc[h],
                            mybir.AluOpType.mult,
                        )
                    return
                # column-split: pipeline the epi multiply with the trailing
                # out-projection so the kernel tail drains block by block
                interleave = list(interleave or [])
                for j in range(KPQ):
                    cs = slice(j * 128, (j + 1) * 128)
                    gs = slice(t * qt + j * 128, t * qt + (j + 1) * 128)
                    for h in range(hl):
                        nc.vector.tensor_tensor(
                            outT_sb[h // 2][64 * (h % 2) : 64 * (h % 2) + 64, gs],
                            u_sb[h][t][:, cs],
                            pbc[h][:, cs],
                            mybir.AluOpType.mult,
                        )
                    if j < len(interleave):
                        interleave[j]()

            # fillers skewed toward the later (longer, ACT-bound) tiles;
            # each tile's epilogue is injected into the next tile's weave
            emit_qkv(0)
            emit_attn_main(0, qkv_groups(1), reserve=2)
            emit_attn_main(1, qkv_groups(2), reserve=2,
                           inject=lambda: emit_epi_head(0))
            emit_attn_main(2, qkv_groups(3) + pso_closures(0), reserve=3,
                           inject=lambda: emit_epi_head(1))
            emit_attn_main(3, pso_closures(1) + pso_closures(2), start=1, reserve=5,
                           inject=lambda: emit_epi_head(2))
            # drain tail: the score slots are free again — rotate the final
            # out-projection through three psum banks
            qrot["pre"] = True
            emit_epi_head(3, col_split=True, interleave=pso_closures(3))

    if cap_waits:
        _cap_matmul_waits(nc)
    return nc


_CAPPED_INSTS = {
    "InstMatmult",
    "InstTensorTensor",
    "InstTensorCopy",
    "InstActivation",
    "InstTensorScalarAffineSelect",
    "InstTensorScalar",
    "InstTensorReduce",
    "InstMemset",
    "InstReciprocal",
    "InstLdweights",
    "InstTensorTensorScan",
    "InstIota",
    "InstDMACopy",
    "InstDrain",
}


def _cap_matmul_waits(nc, max_keep=1):
    """Walrus codegen allows only one sync-wait per compute instruction
    (S3 struct wait slots).  Move excess waits onto NoOps inserted just
    before, on the same engine; engines execute in order so the semantics
    are identical."""
    nop_id = 0
    for f in nc.m.functions:
        for blk in f.blocks:
            insts = blk.instructions
            idx = 0
            while idx < len(insts):
                inst = insts[idx]
                if (
                    type(inst).__name__ in _CAPPED_INSTS
                    and inst.sync_info is not None
                    and len(inst.sync_info.on_wait or []) > max_keep
                ):
                    waits = list(inst.sync_info.on_wait)
                    extra, keep = waits[:-max_keep], waits[-max_keep:]
                    inst.sync_info = mybir.SyncInfo(
                        on_wait=keep, on_update=list(inst.sync_info.on_update or [])
                    )
                    for w in extra:
                        nop = mybir.InstNoOp(name=f"I-mmwait-nop-{nop_id}")
                        nop_id += 1
                        nop.engine = inst.engine
                        nop.sync_info = mybir.SyncInfo(on_wait=[w], on_update=[])
                        insts.insert(idx, nop)
                        idx += 1
                idx += 1


def _rope_tables(n, dh):
    """Host-side RoPE tables in transposed, 2-head-stacked, sign-folded form."""
    inv_freq = 1.0 / (10000.0 ** (np.arange(0, dh, 2, dtype=np.float32) / dh))
    t = np.arange(n, dtype=np.float32)
    freqs = np.outer(t, inv_freq).astype(np.float32)  # [n, dh/2]
    emb = np.concatenate([freqs, freqs], axis=-1)  # [n, dh]
    cos = np.cos(emb).astype(np.float32).T  # [dh, n]
    sin = np.sin(emb).astype(np.float32).T
    sin_signed = sin.copy()
    sin_signed[: dh // 2] *= -1.0
    cosT = np.ascontiguousarray(np.tile(cos, (128 // dh, 1)))
    sinT = np.ascontiguousarray(np.tile(sin_signed, (128 // dh, 1)))
    return cosT, sinT


_NC_CACHE = {}


def kernel(x, w_qkv, w_out):
    return run(x, w_qkv, w_out)[0]


def _bf16(a):
    import ml_dtypes

    return np.asarray(a, dtype=np.float32).astype(ml_dtypes.bfloat16)


def run(x, w_qkv, w_out, trace=False, build_kwargs=None):
    from concourse.bass_utils import run_bass_kernel_spmd

    x = np.asarray(x, dtype=np.float32)
    w_qkv = np.asarray(w_qkv, dtype=np.float32)
    w_out = np.asarray(w_out, dtype=np.float32)

    cosT, sinT = _rope_tables(N, DH)
    # selector for the recip-broadcast matmuls: cols 0:128 map rrA rows
    # {0,32} to heads 0,1 of pbc01; cols 128:192 map row 64 to pbc23[0:64]
    # (head 3 uses a ones vector against rrB)
    selm = np.zeros((65, 192), dtype=np.float32)
    selm[0, 0:64] = 1.0
    selm[32, 64:128] = 1.0
    selm[64, 128:192] = 1.0
    # rotate_half permutation: permM[r, p] = 1 iff r = swap(p)
    permM = np.zeros((128, 128), dtype=np.float32)
    for p in range(128):
        sw = p + 32 if (p % 64) < 32 else p - 32
        permM[sw, p] = 1.0
    in_maps = []
    for core in range(N_CORES):
        b = core // (N_CORES // B)
        g = core % (N_CORES // B)
        cs = slice(g * HL * DH, (g + 1) * HL * DH)
        # x[b].T is [d, n]; reblock into NQT contiguous [d, qt] column blocks
        xt = np.ascontiguousarray(x[b].T)  # [d, n]
        xt_blocks = np.concatenate(
            [xt[:, s * QT : (s + 1) * QT] for s in range(NQT)], axis=0
        )  # [NQT*d, qt]
        in_maps.append(
            {
                "xT": _bf16(xt_blocks),
                "wq": _bf16(w_qkv[:, cs]),
                "wk": _bf16(w_qkv[:, D:][:, cs]),
                "wv": _bf16(w_qkv[:, 2 * D :][:, cs]),
                "wo": _bf16(w_out[cs, :]),
                "cosT": _bf16(cosT),
                "sinT": _bf16(sinT),
                "selc": _bf16(selm),
                "perm": _bf16(permM),
            }
        )

    key = repr(sorted((build_kwargs or {}).items()))
    if key not in _NC_CACHE:
        _NC_CACHE[key] = build_attention_nc(**(build_kwargs or {}))
    nc = _NC_CACHE[key]

    res = run_bass_kernel_spmd(
        nc, in_maps, core_ids=list(range(N_CORES)), trace=trace
    )
    out = np.zeros((B, N, D), dtype=np.float32)
    for core in range(N_CORES):
        out[core // (N_CORES // B)] += np.asarray(
            res.results[core]["out_partial"], dtype=np.float32
        )
    return out, res


if __name__ == "__main__":
    rng = np.random.default_rng(0)
    x = rng.standard_normal((B, N, D), dtype=np.float32)
    w_qkv = rng.standard_normal((D, 3 * D), dtype=np.float32) * D**-0.5
    w_out = rng.standard_normal((D, D), dtype=np.float32) * D**-0.5
    out = kernel(x, w_qkv, w_out)
    print("out", out.shape, out.dtype, float(np.abs(out).max()))


# revision 35
# speedup vs baseline: 1.1370x; 1.0703x over previous
"""Trainium2 Bass kernel for causal multi-head attention with RoPE.

Full module: qkv = x @ w_qkv; RoPE(q, k); causal softmax attention;
out = attn_out @ w_out.  x: [2, 2048, 1024], 16 heads x 64 dim.

Sharding: 8 cores = 2 batches x 4 head-groups (4 heads/core).  Each core
computes its batch's q/k/v for its heads, runs attention, and produces a
partial [2048, 1024] output through its slice of w_out.  Host sums the 4
partials per batch.

v3.1 (on top of v2's st-major software pipeline + bf16 operands):
  - denominator folded into the AV matmul: V stationary is [128, 65]
    with a ones column, so the softmax denominator rides along as psum
    partition 64 (kills the 160 dedicated denominator matmuls, ~29us PE)
  - per-head psum banks av0..av3 [65, qt]; scores keep the [128, 2, qt]
    double-head layout but single-slot (tag scAB, 2 banks); qkv/pso
    double-buffer through wb/qv1
  - RoPE rotate_half via a PE permutation matmul (cross-partition moves
    on DVE/gpsimd are slow in the up direction; the PE does them at
    matmul speed).  The perm matmul for group i is emitted after group
    i+1's qkv matmuls so the PE never waits on the cast.
  - causal diag mask via gpsimd affine_select (no tri mask tensor)
  - softmax recip: den rows gathered by aligned down-copies into rrA
    rows {0,32,64} (heads 0..2) + rrB row 0 (head 3), exp(-ln(d)) on
    ACT, per-head broadcast matmuls into reused av banks
  - diagonal score matmuls restricted to [lo:qt] free range
  - input DMAs issued from three engines in parallel at startup;
    output stores merged to [128, 1024]
"""

import os
import sys

import numpy as np

for _p in ("/opt/trn_rl_repo", "/root/.axon_site/_ro/trn_rl_repo"):
    if os.path.isdir(_p) and _p not in sys.path:
        sys.path.append(_p)

import concourse.bass as bass
import concourse.mybir as mybir
import concourse.tile as tile

F32 = mybir.dt.float32
BF16 = mybir.dt.bfloat16

# Problem constants (hardcoded per contest rules)
B = 2
N = 2048
D = 1024
HEADS = 16
DH = 64
N_CORES = 8
HL = HEADS // (N_CORES // B)  # heads per core = 4

QT = 512
NQT = N // QT        # 4 query tiles
KC = D // 128        # 8 contraction chunks for qkv
CT = (HL * DH) // 128  # 2 column tiles for q/k (2 heads per tile)
NSB = N // 128       # 16 seq blocks
OC = CT              # w_out contraction chunks from this core
KPQ = QT // 128      # key chunks per query tile


def build_attention_nc(qt=QT, lag=5, cap_waits=True):
    n, d, hl, dh = N, D, HL, DH
    nhp = hl // 2
    scale = float(dh) ** -0.5
    nc = bass.Bass("TRN2", target_bir_lowering=False, debug=False)

    xT = nc.dram_tensor("xT", [NQT * d, qt], BF16, kind="ExternalInput").ap()
    wq = nc.dram_tensor("wq", [d, hl * dh], BF16, kind="ExternalInput").ap()
    wk = nc.dram_tensor("wk", [d, hl * dh], BF16, kind="ExternalInput").ap()
    wv = nc.dram_tensor("wv", [d, hl * dh], BF16, kind="ExternalInput").ap()
    wo = nc.dram_tensor("wo", [hl * dh, d], BF16, kind="ExternalInput").ap()
    cosT = nc.dram_tensor("cosT", [128, n], BF16, kind="ExternalInput").ap()
    sinT = nc.dram_tensor("sinT", [128, n], BF16, kind="ExternalInput").ap()
    selc = nc.dram_tensor("selc", [65, 192], BF16, kind="ExternalInput").ap()
    perm = nc.dram_tensor("perm", [128, 128], BF16, kind="ExternalInput").ap()
    outp = nc.dram_tensor("out_partial", [n, d], BF16, kind="ExternalOutput").ap()

    with tile.TileContext(nc) as tc:
        with (
            tc.tile_pool(name="pers", bufs=1) as pers,
            tc.tile_pool(name="ps", bufs=1, space="PSUM") as ps,
            tc.tile_pool(name="ropet", bufs=4) as ropet,
            tc.tile_pool(name="expp", bufs=12) as expp,
            tc.tile_pool(name="bcp", bufs=4) as bcp,
            tc.tile_pool(name="fo", bufs=4) as fo,
        ):
            # ---- persistent SBUF ----
            x_sb = [
                pers.tile([128, KC, qt], BF16, tag=f"x{s}", name=f"x{s}")
                for s in range(NQT)
            ]
            wq_sb = pers.tile([128, KC, hl * dh], BF16, tag="wq", name="wq")
            wk_sb = pers.tile([128, KC, hl * dh], BF16, tag="wk", name="wk")
            wv_sb = pers.tile([128, KC, hl * dh], BF16, tag="wv", name="wv")
            wo_sb = pers.tile([128, OC, d], BF16, tag="wo", name="wo")
            cos_sb = pers.tile([128, n], BF16, tag="cos", name="cos")
            sin_sb = pers.tile([128, n], BF16, tag="sin", name="sin")
            perm_sb = pers.tile([128, 128], BF16, tag="perm", name="perm")
            qT_sb = [pers.tile([128, n], BF16, tag=f"qT{i}", name=f"qT{i}") for i in range(CT)]
            kT_sb = [pers.tile([128, n], BF16, tag=f"kT{i}", name=f"kT{i}") for i in range(CT)]
            v_sb = [
                pers.tile([128, hl, dh + 1], BF16, tag=f"v{i}", name=f"v{i}")
                for i in range(NSB)
            ]
            outT_sb = [pers.tile([128, n], BF16, tag=f"oT{i}", name=f"oT{i}") for i in range(CT)]
            # unnormalized AV per (head, tile)
            u_sb = [
                [pers.tile([64, qt], F32, tag=f"u{h}_{t}", name=f"u{h}_{t}") for t in range(NQT)]
                for h in range(hl)
            ]
            # den gather rows: heads 0..2 at rrA rows {0,32,64}, head 3 at rrB
            rrA_f = pers.tile([65, qt], F32, tag="rrAf", name="rrAf")
            rrB_f = pers.tile([1, qt], F32, tag="rrBf", name="rrBf")
            # selector matrices (host-built): cols 0:128 broadcast rrA rows
            # {0,32} onto pbc01; cols 128:192 broadcast row 64 onto pbc23[0:64]
            selc_sb = pers.tile([65, 192], BF16, tag="selc", name="selc")
            ones1_sb = pers.tile([1, 64], BF16, tag="ones1", name="ones1")

            # ---- setup: v ones columns + rrA neutral fill + sel ones ----
            for sb in range(NSB):
                nc.vector.memset(v_sb[sb][:, :, dh : dh + 1], 1.0)
            nc.vector.memset(rrA_f, 1.0)
            nc.vector.memset(ones1_sb, 1.0)
            nc.gpsimd.dma_start(selc_sb, selc)
            nc.gpsimd.dma_start(perm_sb, perm)

            # ---- input DMA (bf16, host pre-cast): ALL on sync, in strict
            # priority order — the queues drain roughly FIFO, so issue order
            # IS arrival order.  Matches the qkv(0) group order
            # q0,q1,k0,k1,v0..v3.
            hd = d // 2
            qd = d // 4
            nc.sync.dma_start(
                wq_sb[:, 0 : KC // 2, :],
                wq[0:hd, :].rearrange("(kc p) m -> p kc m", p=128),
            )
            for quarter in range(4):
                sl = slice(quarter * qd, (quarter + 1) * qd)
                nc.sync.dma_start(
                    x_sb[0][:, quarter * KC // 4 : (quarter + 1) * KC // 4, :],
                    xT[sl, :].rearrange("(kc p) m -> p kc m", p=128),
                )
            nc.sync.dma_start(
                wq_sb[:, KC // 2 : KC, :],
                wq[hd:d, :].rearrange("(kc p) m -> p kc m", p=128),
            )
            nc.sync.dma_start(wk_sb, wk.rearrange("(kc p) m -> p kc m", p=128))
            nc.sync.dma_start(cos_sb, cosT)
            nc.sync.dma_start(sin_sb, sinT)
            nc.sync.dma_start(wv_sb, wv.rearrange("(kc p) m -> p kc m", p=128))
            for s in range(1, NQT):
                nc.sync.dma_start(
                    x_sb[s],
                    xT[s * d : (s + 1) * d, :].rearrange("(kc p) m -> p kc m", p=128),
                )
            nc.sync.dma_start(wo_sb, wo.rearrange("(kc p) m -> p kc m", p=128))

            # Filler psum: single bank "wb" once attention starts (the score
            # slots need 4 banks).  During qkv(0) — before any scores — the
            # idle score slots serve as extra qkv buffers.
            qrot = {"i": 0, "pre": True}

            def alloc_fill(width, name):
                """Allocate a [128, width] psum region for qkv/pso work."""
                if qrot["pre"]:
                    j = qrot["i"] % 3
                    qrot["i"] += 1
                    if j > 0:
                        t = ps.tile([128, 2, qt], F32, tag=f"sc{'AB'[j - 1]}", name=name)
                        return t[:, 0, 0:width]
                t = ps.tile([128, width], F32, tag="wb", name=name)
                return t

            def qkv_groups(st):
                """Emit closures: qkv matmul groups with the RoPE tail of
                group i emitted after the matmuls of group i+1 (the perm
                matmul then never stalls the PE on the psum cast)."""
                qsl = slice(st * qt, (st + 1) * qt)
                ems = []

                def qk_mm(ct, qk, w_sb):
                    pq = alloc_fill(qt, f"pq{st}_{ct}_{qk}")
                    for kc in range(KC):
                        nc.tensor.matmul(
                            pq,
                            w_sb[:, kc, ct * 128 : (ct + 1) * 128],
                            x_sb[st][:, kc, :],
                            start=(kc == 0),
                            stop=(kc == KC - 1),
                        )
                    # raw (bf16, for the perm matmul) and the cos product are
                    # both read straight from psum; pq's lifetime stays inside
                    # this group so the wb/qv1 rotation remains safe
                    raw = ropet.tile([128, qt], BF16, tag="raw", name="raw")
                    nc.vector.tensor_copy(raw, pq)
                    a = ropet.tile([128, qt], BF16, tag="a", name="a")
                    nc.vector.tensor_tensor(a, pq, cos_sb[:, qsl], mybir.AluOpType.mult)
                    return raw, a

                def rope_tail(raw, a, ct, dst):
                    # rawS = perm @ raw on the PE (32-partition block swap);
                    # sin product reads the psum result directly
                    psP = alloc_fill(qt, "psP")
                    nc.tensor.matmul(psP, perm_sb, raw, start=True, stop=True)
                    sh = ropet.tile([128, qt], BF16, tag="sh", name="sh")
                    nc.vector.tensor_tensor(sh, psP, sin_sb[:, qsl], mybir.AluOpType.mult)
                    nc.gpsimd.tensor_tensor(dst[ct][:, qsl], a, sh, mybir.AluOpType.add)

                def v_group(j):
                    sb = st * KPQ + j
                    psv = alloc_fill(hl * dh, f"psv{sb}")
                    for kc in range(KC):
                        nc.tensor.matmul(
                            psv,
                            x_sb[st][:, kc, j * 128 : (j + 1) * 128],
                            wv_sb[:, kc, :],
                            start=(kc == 0),
                            stop=(kc == KC - 1),
                        )
                    nc.vector.tensor_copy(
                        v_sb[sb][:, :, 0:dh], psv.rearrange("p (h e) -> p h e", h=hl)
                    )

                # interleave: mm(i), mm(i+1), tail(i), mm(i+2), tail(i+1), ...
                pend = []  # (raw, a, ct, dst) awaiting tail

                def mk_mm(ct, qk, w_sb, dst):
                    def em():
                        raw, a = qk_mm(ct, qk, w_sb)
                        pend.append((raw, a, ct, dst))

                    return em

                def mk_tail():
                    def em():
                        raw, a, ct, dst = pend.pop(0)
                        rope_tail(raw, a, ct, dst)

                    return em

                qks = []
                for qk, w_sb, dst in ((0, wq_sb, qT_sb), (1, wk_sb, kT_sb)):
                    for ct in range(CT):
                        qks.append(mk_mm(ct, qk, w_sb, dst))
                ems.append(qks[0])
                ems.append(qks[1])
                ems.append(mk_tail())
                ems.append(qks[2])
                ems.append(mk_tail())
                ems.append(qks[3])
                ems.append(mk_tail())
                ems.append(lambda: v_group(0))
                ems.append(mk_tail())
                for j in range(1, KPQ):
                    ems.append(lambda j=j: v_group(j))
                return ems

            def emit_qkv(st):
                for em in qkv_groups(st):
                    em()

            attn_state = {}

            def emit_attn_main(t, fillers=(), start=0, reserve=0, inject=None):
                fillers = list(fillers)
                spread = fillers[: len(fillers) - reserve]
                reserved = fillers[len(fillers) - reserve :]
                qrot["pre"] = False
                qsl = slice(t * qt, (t + 1) * qt)
                ncc = KPQ * (t + 1)
                # AV/den psum banks: av01 pair-packs heads 0,1 [128, qt];
                # av2/av3 hold heads 2,3 as [0:64) AV + row 64 den, plus the
                # dens of heads 0,1 parked at row 96 by dedicated den matmuls.
                # Allocated lazily (first AV use) so the previous tile's
                # epilogue pbc tiles can be injected into this tile's weave
                # without inverting the av-bank tag order.
                pav = {}
                e_ts = {}
                scnt = {"i": 0}

                def emit_scores(c, hp):
                    j = c - KPQ * t
                    lo = max(0, j * 128)
                    pss = ps.tile(
                        [128, 2, qt], F32, tag=f"sc{'AB'[scnt['i'] % 2]}",
                        name=f"pss{t}_{c}_{hp}",
                    )
                    scnt["i"] += 1
                    for g in range(2):
                        bp = 64 * g
                        nc.tensor.matmul(
                            pss[:, g, lo:qt],
                            kT_sb[hp][bp : bp + dh, c * 128 : (c + 1) * 128],
                            qT_sb[hp][bp : bp + dh, t * qt + lo : (t + 1) * qt],
                            start=True,
                            stop=True,
                        )
                    e_t = expp.tile([128, 2, qt], BF16, tag="e", name="e")
                    nc.scalar.activation(
                        e_t[:, :, lo:qt], pss[:, :, lo:qt],
                        mybir.ActivationFunctionType.Exp, scale=scale,
                    )
                    if j >= 0:
                        # causal mask on the diagonal block: keep where
                        # query offset >= key offset
                        nc.gpsimd.affine_select(
                            out=e_t[:, :, lo : lo + 128],
                            in_=e_t[:, :, lo : lo + 128],
                            compare_op=mybir.AluOpType.is_ge,
                            fill=0.0,
                            base=0,
                            channel_multiplier=-1,
                            pattern=[[0, 2], [1, 128]],
                        )
                    e_ts[(c, hp)] = e_t

                def get_pav():
                    if "01" not in pav:
                        pav["01"] = ps.tile([128, qt], F32, tag="av01", name=f"pav{t}_01")
                        pav["2"] = ps.tile([97, qt], F32, tag="av2", name=f"pav{t}_2")
                        pav["3"] = ps.tile([97, qt], F32, tag="av3", name=f"pav{t}_3")
                    return pav

                def emit_av(c, hp):
                    lo = max(0, (c - KPQ * t) * 128)
                    e_t = e_ts.pop((c, hp))
                    p = get_pav()
                    st_fl = (c == 0)
                    sp_fl = (c == ncc - 1)
                    if hp == 0:
                        for g in range(2):
                            nc.tensor.matmul(
                                p["01"][64 * g : 64 * g + 64, lo:qt],
                                v_sb[c][:, g, 0:dh],
                                e_t[:, g, lo:qt],
                                start=st_fl, stop=sp_fl,
                                tile_position=(0, 64 * g),
                            )
                        # dens of heads 0,1 ride at row 96 of av2/av3
                        for g in range(2):
                            nc.tensor.matmul(
                                p[str(2 + g)][96:97, lo:qt],
                                v_sb[c][:, g, dh : dh + 1],
                                e_t[:, g, lo:qt],
                                start=st_fl, stop=sp_fl,
                                tile_position=(0, 96),
                            )
                    else:
                        for g in range(2):
                            nc.tensor.matmul(
                                p[str(2 + g)][0:65, lo:qt],
                                v_sb[c][:, 2 + g, :],
                                e_t[:, g, lo:qt],
                                start=st_fl, stop=sp_fl,
                            )

                done = 0
                nspread = len(spread)

                def advance(frac):
                    # weave filler groups: the PE keeps dense work that
                    # doesn't depend on ACT's exp stream
                    nonlocal done
                    if nspread:
                        want = min(nspread, int(frac * nspread))
                        while done < want:
                            spread[done]()
                            done += 1

                for c in range(ncc):
                    # interleave AV matmuls between the two serialized score
                    # pairs so the PE has work while ACT runs each exp
                    emit_scores(c, 0)
                    if c >= lag:
                        emit_av(c - lag, 0)
                    if c >= start:
                        advance((c - start + 0.5) / max(1, ncc - start))
                    emit_scores(c, 1)
                    if c >= lag:
                        emit_av(c - lag, 1)
                    if inject is not None and c == 1:
                        # previous tile's epilogue: its recip chain overlaps
                        # this tile's early chunks instead of blocking the PE
                        inject()
                        inject = None
                    if c >= start:
                        advance((c - start + 1.0) / max(1, ncc - start))
                for c in range(max(0, ncc - lag), ncc):
                    emit_av(c, 0)
                    emit_av(c, 1)
                advance(1.0)

                # softmax denominator path: aligned down-copies of the den
                # rows into rrA rows {0,32,64} (h=0..2) and rrB row 0 (h=3);
                # recip = exp(-ln(d)) on ACT
                p = get_pav()
                # on the last tile ACT's exp stream is done: split the gather
                # and u copies across DVE and ACT to halve the tail chain
                alt = nc.scalar.copy if t == NQT - 1 else nc.vector.tensor_copy
                nc.vector.tensor_copy(rrA_f[0:1, :], p["2"][96:97, :])
                alt(rrA_f[32:33, :], p["3"][96:97, :])
                nc.vector.tensor_copy(rrA_f[64:65, :], p["2"][64:65, :])
                alt(rrB_f, p["3"][64:65, :])
                lnA = bcp.tile([65, qt], F32, tag="lnA", name="lnA")
                lnB = bcp.tile([1, qt], F32, tag="lnB", name="lnB")
                nc.scalar.activation(lnA, rrA_f, mybir.ActivationFunctionType.Ln)
                nc.scalar.activation(lnB, rrB_f, mybir.ActivationFunctionType.Ln)
                rrA = bcp.tile([65, qt], BF16, tag="rrA", name="rrA")
                rrB = bcp.tile([1, qt], BF16, tag="rrB", name="rrB")
                with nc.allow_low_precision(reason="bf16 softmax recip"):
                    nc.scalar.activation(
                        rrA, lnA, mybir.ActivationFunctionType.Exp, scale=-1.0
                    )
                    nc.scalar.activation(
                        rrB, lnB, mybir.ActivationFunctionType.Exp, scale=-1.0
                    )
                attn_state[t] = (rrA, rrB)

                # unnormalized AV -> SBUF, per head (aligned/down copies)
                nc.vector.tensor_copy(u_sb[0][t], p["01"][0:64, :])
                alt(u_sb[1][t], p["01"][64:128, :])
                nc.vector.tensor_copy(u_sb[2][t], p["2"][0:64, :])
                alt(u_sb[3][t], p["3"][0:64, :])

                # reserved fillers keep the PE busy while the recip chain runs
                for f in reserved:
                    f()

            def pso_closures(t):
                ems = []
                for j in range(KPQ):
                    sb = t * KPQ + j

                    def em(sb=sb):
                        o_t = fo.tile([128, 2, 512], BF16, tag="ot", name="ot")
                        for nt in range(2):
                            pso = alloc_fill(512, f"pso{sb}_{nt}")
                            for kc in range(OC):
                                nc.tensor.matmul(
                                    pso,
                                    outT_sb[kc][:, sb * 128 : (sb + 1) * 128],
                                    wo_sb[:, kc, nt * 512 : (nt + 1) * 512],
                                    start=(kc == 0),
                                    stop=(kc == OC - 1),
                                )
                            # in the drain tail (pre-mode), ACT is idle:
                            # alternate the psum->sbuf copies across engines
                            if qrot["pre"] and nt == 1:
                                nc.scalar.copy(o_t[:, nt, :], pso)
                            else:
                                nc.vector.tensor_copy(o_t[:, nt, :], pso)
                        nc.sync.dma_start(
                            outp[sb * 128 : (sb + 1) * 128, :],
                            o_t.rearrange("p a b -> p (a b)"),
                        )

                    ems.append(em)
                return ems

            def emit_epi_head(t, col_split=False, interleave=None):
                qsl = slice(t * qt, (t + 1) * qt)
                rrA, rrB = attn_state.pop(t)
                # broadcast recips: pbc01 pair-packs heads 0,1; pbc23 heads
                # 2,3 (head 3 via a ones-vector matmul into partitions 64+)
                pbc01 = ps.tile([128, qt], F32, tag="av01", name=f"pbc{t}_01")
                nc.tensor.matmul(pbc01, selc_sb[:, 0:128], rrA, start=True, stop=True)
                pbc23 = ps.tile([128, qt], F32, tag="av2", name=f"pbc{t}_23")
                nc.tensor.matmul(
                    pbc23[0:64, :], selc_sb[:, 128:192], rrA, start=True, stop=True
                )
                nc.tensor.matmul(
                    pbc23[64:128, :], ones1_sb, rrB, start=True, stop=True,
                    tile_position=(0, 64),
                )
                pbc = [pbc01[0:64, :], pbc01[64:128, :], pbc23[0:64, :], pbc23[64:128, :]]
                if not col_split:
                    for h in range(hl):
                        nc.vector.tensor_tensor(
                            outT_sb[h // 2][64 * (h % 2) : 64 * (h % 2) + 64, qsl],
                            u_sb[h][t],
                            pbc[h],
                            mybir.AluOpType.mult,
                        )
                    return
                # column-split: pipeline the epi multiply with the trailing
                # out-projection so the kernel tail drains block by block
                interleave = list(interleave or [])
                for j in range(KPQ):
                    cs = slice(j * 128, (j + 1) * 128)
                    gs = slice(t * qt + j * 128, t * qt + (j + 1) * 128)
                    for h in range(hl):
                        nc.vector.tensor_tensor(
                            outT_sb[h // 2][64 * (h % 2) : 64 * (h % 2) + 64, gs],
                            u_sb[h][t][:, cs],
                            pbc[h][:, cs],
                            mybir.AluOpType.mult,
                        )
                    if j < len(interleave):
                        interleave[j]()

            # fillers skewed toward the later (longer, ACT-bound) tiles;
            # each tile's epilogue is injected into the next tile's weave
            emit_qkv(0)
            emit_attn_main(0, qkv_groups(1), reserve=2)
            emit_attn_main(1, qkv_groups(2), reserve=2,
                           inject=lambda: emit_epi_head(0))
            emit_attn_main(2, qkv_groups(3) + pso_closures(0), reserve=3,
                           inject=lambda: emit_epi_head(1))
            emit_attn_main(3, pso_closures(1) + pso_closures(2), start=1, reserve=5,
                           inject=lambda: emit_epi_head(2))
            # drain tail: the score slots are free again — rotate the final
            # out-projection through three psum banks
            qrot["pre"] = True
            emit_epi_head(3, col_split=True, interleave=pso_closures(3))

    if cap_waits:
        _cap_matmul_waits(nc)
    return nc


_CAPPED_INSTS = {
    "InstMatmult",
    "InstTensorTensor",
    "InstTensorCopy",
    "InstActivation",
    "InstTensorScalarAffineSelect",
    "InstTensorScalar",
    "InstTensorReduce",
    "InstMemset",
    "InstReciprocal",
    "InstLdweights",
    "InstTensorTensorScan",
    "InstIota",
    "InstDMACopy",
    "InstDrain",
}


def _cap_matmul_waits(nc, max_keep=1):
    """Walrus codegen allows only one sync-wait per compute instruction
    (S3 struct wait slots).  Move excess waits onto NoOps inserted just
    before, on the same engine; engines execute in order so the semantics
    are identical."""
    nop_id = 0
    for f in nc.m.functions:
        for blk in f.blocks:
            insts = blk.instructions
            idx = 0
            while idx < len(insts):
                inst = insts[idx]
                if (
                    type(inst).__name__ in _CAPPED_INSTS
                    and inst.sync_info is not None
                    and len(inst.sync_info.on_wait or []) > max_keep
                ):
                    waits = list(inst.sync_info.on_wait)
                    extra, keep = waits[:-max_keep], waits[-max_keep:]
                    inst.sync_info = mybir.SyncInfo(
                        on_wait=keep, on_update=list(inst.sync_info.on_update or [])
                    )
                    for w in extra:
                        nop = mybir.InstNoOp(name=f"I-mmwait-nop-{nop_id}")
                        nop_id += 1
                        nop.engine = inst.engine
                        nop.sync_info = mybir.SyncInfo(on_wait=[w], on_update=[])
                        insts.insert(idx, nop)
                        idx += 1
                idx += 1


def _rope_tables(n, dh):
    """Host-side RoPE tables in transposed, 2-head-stacked, sign-folded form."""
    inv_freq = 1.0 / (10000.0 ** (np.arange(0, dh, 2, dtype=np.float32) / dh))
    t = np.arange(n, dtype=np.float32)
    freqs = np.outer(t, inv_freq).astype(np.float32)  # [n, dh/2]
    emb = np.concatenate([freqs, freqs], axis=-1)  # [n, dh]
    cos = np.cos(emb).astype(np.float32).T  # [dh, n]
    sin = np.sin(emb).astype(np.float32).T
    sin_signed = sin.copy()
    sin_signed[: dh // 2] *= -1.0
    cosT = np.ascontiguousarray(np.tile(cos, (128 // dh, 1)))
    sinT = np.ascontiguousarray(np.tile(sin_signed, (128 // dh, 1)))
    return cosT, sinT


_NC_CACHE = {}


def kernel(x, w_qkv, w_out):
    return run(x, w_qkv, w_out)[0]


def _bf16(a):
    import ml_dtypes

    return np.asarray(a, dtype=np.float32).astype(ml_dtypes.bfloat16)


def run(x, w_qkv, w_out, trace=False, build_kwargs=None):
    from concourse.bass_utils import run_bass_kernel_spmd

    x = np.asarray(x, dtype=np.float32)
    w_qkv = np.asarray(w_qkv, dtype=np.float32)
    w_out = np.asarray(w_out, dtype=np.float32)

    cosT, sinT = _rope_tables(N, DH)
    # selector for the recip-broadcast matmuls: cols 0:128 map rrA rows
    # {0,32} to heads 0,1 of pbc01; cols 128:192 map row 64 to pbc23[0:64]
    # (head 3 uses a ones vector against rrB)
    selm = np.zeros((65, 192), dtype=np.float32)
    selm[0, 0:64] = 1.0
    selm[32, 64:128] = 1.0
    selm[64, 128:192] = 1.0
    # rotate_half permutation: permM[r, p] = 1 iff r = swap(p)
    permM = np.zeros((128, 128), dtype=np.float32)
    for p in range(128):
        sw = p + 32 if (p % 64) < 32 else p - 32
        permM[sw, p] = 1.0
    in_maps = []
    for core in range(N_CORES):
        b = core // (N_CORES // B)
        g = core % (N_CORES // B)
        cs = slice(g * HL * DH, (g + 1) * HL * DH)
        # x[b].T is [d, n]; reblock into NQT contiguous [d, qt] column blocks
        xt = np.ascontiguousarray(x[b].T)  # [d, n]
        xt_blocks = np.concatenate(
            [xt[:, s * QT : (s + 1) * QT] for s in range(NQT)], axis=0
        )  # [NQT*d, qt]
        in_maps.append(
            {
                "xT": _bf16(xt_blocks),
                "wq": _bf16(w_qkv[:, cs]),
                "wk": _bf16(w_qkv[:, D:][:, cs]),
                "wv": _bf16(w_qkv[:, 2 * D :][:, cs]),
                "wo": _bf16(w_out[cs, :]),
                "cosT": _bf16(cosT),
                "sinT": _bf16(sinT),
                "selc": _bf16(selm),
                "perm": _bf16(permM),
            }
        )

    key = repr(sorted((build_kwargs or {}).items()))
    if key not in _NC_CACHE:
        _NC_CACHE[key] = build_attention_nc(**(build_kwargs or {}))
    nc = _NC_CACHE[key]

    res = run_bass_kernel_spmd(
        nc, in_maps, core_ids=list(range(N_CORES)), trace=trace
    )
    out = np.zeros((B, N, D), dtype=np.float32)
    for core in range(N_CORES):
        out[core // (N_CORES // B)] += np.asarray(
            res.results[core]["out_partial"], dtype=np.float32
        )
    return out, res


if __name__ == "__main__":
    rng = np.random.default_rng(0)
    x = rng.standard_normal((B, N, D), dtype=np.float32)
    w_qkv = rng.standard_normal((D, 3 * D), dtype=np.float32) * D**-0.5
    w_out = rng.standard_normal((D, D), dtype=np.float32) * D**-0.5
    out = kernel(x, w_qkv, w_out)
    print("out", out.shape, out.dtype, float(np.abs(out).max()))


# revision 38
# speedup vs baseline: 1.1882x; 1.0451x over previous
"""Trainium2 Bass kernel for causal multi-head attention with RoPE.

Full module: qkv = x @ w_qkv; RoPE(q, k); causal softmax attention;
out = attn_out @ w_out.  x: [2, 2048, 1024], 16 heads x 64 dim.

Sharding: 8 cores = 2 batches x 4 head-groups (4 heads/core).  Each core
computes its batch's q/k/v for its heads, runs attention, and produces a
partial [2048, 1024] output through its slice of w_out.  Host sums the 4
partials per batch.

v3.1 (on top of v2's st-major software pipeline + bf16 operands):
  - denominator folded into the AV matmul: V stationary is [128, 65]
    with a ones column, so the softmax denominator rides along as psum
    partition 64 (kills the 160 dedicated denominator matmuls, ~29us PE)
  - per-head psum banks av0..av3 [65, qt]; scores keep the [128, 2, qt]
    double-head layout but single-slot (tag scAB, 2 banks); qkv/pso
    double-buffer through wb/qv1
  - RoPE rotate_half via a PE permutation matmul (cross-partition moves
    on DVE/gpsimd are slow in the up direction; the PE does them at
    matmul speed).  The perm matmul for group i is emitted after group
    i+1's qkv matmuls so the PE never waits on the cast.
  - causal diag mask via gpsimd affine_select (no tri mask tensor)
  - softmax recip: den rows gathered by aligned down-copies into rrA
    rows {0,32,64} (heads 0..2) + rrB row 0 (head 3), exp(-ln(d)) on
    ACT, per-head broadcast matmuls into reused av banks
  - diagonal score matmuls restricted to [lo:qt] free range
  - input DMAs issued from three engines in parallel at startup;
    output stores merged to [128, 1024]
"""

import os
import sys

import numpy as np

for _p in ("/opt/trn_rl_repo", "/root/.axon_site/_ro/trn_rl_repo"):
    if os.path.isdir(_p) and _p not in sys.path:
        sys.path.append(_p)

import concourse.bass as bass
import concourse.mybir as mybir
import concourse.tile as tile

F32 = mybir.dt.float32
BF16 = mybir.dt.bfloat16

# Problem constants (hardcoded per contest rules)
B = 2
N = 2048
D = 1024
HEADS = 16
DH = 64
N_CORES = 8
HL = HEADS // (N_CORES // B)  # heads per core = 4

QT = 512
NQT = N // QT        # 4 query tiles
KC = D // 128        # 8 contraction chunks for qkv
CT = (HL * DH) // 128  # 2 column tiles for q/k (2 heads per tile)
NSB = N // 128       # 16 seq blocks
OC = CT              # w_out contraction chunks from this core
KPQ = QT // 128      # key chunks per query tile


def build_attention_nc(qt=QT, lag=5, cap_waits=True):
    n, d, hl, dh = N, D, HL, DH
    nhp = hl // 2
    scale = float(dh) ** -0.5
    nc = bass.Bass("TRN2", target_bir_lowering=False, debug=False)

    xT = nc.dram_tensor("xT", [NQT * d, qt], BF16, kind="ExternalInput").ap()
    wq = nc.dram_tensor("wq", [d, hl * dh], BF16, kind="ExternalInput").ap()
    wk = nc.dram_tensor("wk", [d, hl * dh], BF16, kind="ExternalInput").ap()
    wv = nc.dram_tensor("wv", [d, hl * dh], BF16, kind="ExternalInput").ap()
    wo = nc.dram_tensor("wo", [hl * dh, d], BF16, kind="ExternalInput").ap()
    cosT = nc.dram_tensor("cosT", [128, n], BF16, kind="ExternalInput").ap()
    sinT = nc.dram_tensor("sinT", [128, n], BF16, kind="ExternalInput").ap()
    selc = nc.dram_tensor("selc", [65, 192], BF16, kind="ExternalInput").ap()
    perm = nc.dram_tensor("perm", [128, 128], BF16, kind="ExternalInput").ap()
    outp = nc.dram_tensor("out_partial", [n, d], BF16, kind="ExternalOutput").ap()

    with tile.TileContext(nc) as tc:
        with (
            tc.tile_pool(name="pers", bufs=1) as pers,
            tc.tile_pool(name="ps", bufs=1, space="PSUM") as ps,
            tc.tile_pool(name="ropet", bufs=4) as ropet,
            tc.tile_pool(name="expp", bufs=12) as expp,
            tc.tile_pool(name="bcp", bufs=4) as bcp,
            tc.tile_pool(name="fo", bufs=4) as fo,
        ):
            # ---- persistent SBUF ----
            x_sb = [
                pers.tile([128, KC, qt], BF16, tag=f"x{s}", name=f"x{s}")
                for s in range(NQT)
            ]
            wq_sb = pers.tile([128, KC, hl * dh], BF16, tag="wq", name="wq")
            wk_sb = pers.tile([128, KC, hl * dh], BF16, tag="wk", name="wk")
            wv_sb = pers.tile([128, KC, hl * dh], BF16, tag="wv", name="wv")
            wo_sb = pers.tile([128, OC, d], BF16, tag="wo", name="wo")
            cos_sb = pers.tile([128, n], BF16, tag="cos", name="cos")
            sin_sb = pers.tile([128, n], BF16, tag="sin", name="sin")
            perm_sb = pers.tile([128, 128], BF16, tag="perm", name="perm")
            qT_sb = [pers.tile([128, n], BF16, tag=f"qT{i}", name=f"qT{i}") for i in range(CT)]
            kT_sb = [pers.tile([128, n], BF16, tag=f"kT{i}", name=f"kT{i}") for i in range(CT)]
            v_sb = [
                pers.tile([128, hl, dh + 1], BF16, tag=f"v{i}", name=f"v{i}")
                for i in range(NSB)
            ]
            outT_sb = [pers.tile([128, n], BF16, tag=f"oT{i}", name=f"oT{i}") for i in range(CT)]
            # unnormalized AV per (head, tile)
            u_sb = [
                [pers.tile([64, qt], F32, tag=f"u{h}_{t}", name=f"u{h}_{t}") for t in range(NQT)]
                for h in range(hl)
            ]
            # den gather rows: heads 0..2 at rrA rows {0,32,64}, head 3 at rrB
            rrA_f = pers.tile([65, qt], F32, tag="rrAf", name="rrAf")
            rrB_f = pers.tile([1, qt], F32, tag="rrBf", name="rrBf")
            # selector matrices (host-built): cols 0:128 broadcast rrA rows
            # {0,32} onto pbc01; cols 128:192 broadcast row 64 onto pbc23[0:64]
            selc_sb = pers.tile([65, 192], BF16, tag="selc", name="selc")
            ones1_sb = pers.tile([1, 64], BF16, tag="ones1", name="ones1")

            # ---- setup: v ones columns + rrA neutral fill + sel ones ----
            for sb in range(NSB):
                nc.vector.memset(v_sb[sb][:, :, dh : dh + 1], 1.0)
            nc.vector.memset(rrA_f, 1.0)
            nc.vector.memset(ones1_sb, 1.0)
            nc.gpsimd.dma_start(selc_sb, selc)
            nc.gpsimd.dma_start(perm_sb, perm)

            # ---- input DMA (bf16, host pre-cast): ALL on sync, in strict
            # priority order — the queues drain roughly FIFO, so issue order
            # IS arrival order.  Matches the qkv(0) group order
            # q0,q1,k0,k1,v0..v3.
            hd = d // 2
            qd = d // 4
            nc.sync.dma_start(
                wq_sb[:, 0 : KC // 2, :],
                wq[0:hd, :].rearrange("(kc p) m -> p kc m", p=128),
            )
            for quarter in range(4):
                sl = slice(quarter * qd, (quarter + 1) * qd)
                nc.sync.dma_start(
                    x_sb[0][:, quarter * KC // 4 : (quarter + 1) * KC // 4, :],
                    xT[sl, :].rearrange("(kc p) m -> p kc m", p=128),
                )
            nc.sync.dma_start(
                wq_sb[:, KC // 2 : KC, :],
                wq[hd:d, :].rearrange("(kc p) m -> p kc m", p=128),
            )
            nc.sync.dma_start(wk_sb, wk.rearrange("(kc p) m -> p kc m", p=128))
            nc.sync.dma_start(cos_sb, cosT)
            nc.sync.dma_start(sin_sb, sinT)
            nc.sync.dma_start(wv_sb, wv.rearrange("(kc p) m -> p kc m", p=128))
            for s in range(1, NQT):
                nc.sync.dma_start(
                    x_sb[s],
                    xT[s * d : (s + 1) * d, :].rearrange("(kc p) m -> p kc m", p=128),
                )
            nc.sync.dma_start(wo_sb, wo.rearrange("(kc p) m -> p kc m", p=128))

            # Filler psum: single bank "wb" once attention starts (the score
            # slots need 4 banks).  During qkv(0) — before any scores — the
            # idle score slots serve as extra qkv buffers.
            qrot = {"i": 0, "pre": True}

            def alloc_fill(width, name):
                """Allocate a [128, width] psum region for qkv/pso work."""
                if qrot["pre"]:
                    j = qrot["i"] % 3
                    qrot["i"] += 1
                    if j > 0:
                        t = ps.tile([128, 2, qt], F32, tag=f"sc{'AB'[j - 1]}", name=name)
                        return t[:, 0, 0:width]
                t = ps.tile([128, width], F32, tag="wb", name=name)
                return t

            def qkv_groups(st):
                """Emit closures: qkv matmul groups with the RoPE tail of
                group i emitted after the matmuls of group i+1 (the perm
                matmul then never stalls the PE on the psum cast)."""
                qsl = slice(st * qt, (st + 1) * qt)
                ems = []

                def qk_mm(ct, qk, w_sb):
                    pq = alloc_fill(qt, f"pq{st}_{ct}_{qk}")
                    for kc in range(KC):
                        nc.tensor.matmul(
                            pq,
                            w_sb[:, kc, ct * 128 : (ct + 1) * 128],
                            x_sb[st][:, kc, :],
                            start=(kc == 0),
                            stop=(kc == KC - 1),
                        )
                    # raw (bf16, for the perm matmul) and the cos product are
                    # both read straight from psum; pq's lifetime stays inside
                    # this group so the wb/qv1 rotation remains safe
                    raw = ropet.tile([128, qt], BF16, tag="raw", name="raw")
                    nc.vector.tensor_copy(raw, pq)
                    a = ropet.tile([128, qt], BF16, tag="a", name="a")
                    nc.vector.tensor_tensor(a, pq, cos_sb[:, qsl], mybir.AluOpType.mult)
                    return raw, a

                def rope_tail(raw, a, ct, dst):
                    # rawS = perm @ raw on the PE (32-partition block swap);
                    # sin product reads the psum result directly
                    psP = alloc_fill(qt, "psP")
                    nc.tensor.matmul(psP, perm_sb, raw, start=True, stop=True)
                    sh = ropet.tile([128, qt], BF16, tag="sh", name="sh")
                    nc.vector.tensor_tensor(sh, psP, sin_sb[:, qsl], mybir.AluOpType.mult)
                    nc.gpsimd.tensor_tensor(dst[ct][:, qsl], a, sh, mybir.AluOpType.add)

                def v_group(j):
                    sb = st * KPQ + j
                    psv = alloc_fill(hl * dh, f"psv{sb}")
                    for kc in range(KC):
                        nc.tensor.matmul(
                            psv,
                            x_sb[st][:, kc, j * 128 : (j + 1) * 128],
                            wv_sb[:, kc, :],
                            start=(kc == 0),
                            stop=(kc == KC - 1),
                        )
                    nc.vector.tensor_copy(
                        v_sb[sb][:, :, 0:dh], psv.rearrange("p (h e) -> p h e", h=hl)
                    )

                # interleave: mm(i), mm(i+1), tail(i), mm(i+2), tail(i+1), ...
                pend = []  # (raw, a, ct, dst) awaiting tail

                def mk_mm(ct, qk, w_sb, dst):
                    def em():
                        raw, a = qk_mm(ct, qk, w_sb)
                        pend.append((raw, a, ct, dst))

                    return em

                def mk_tail():
                    def em():
                        raw, a, ct, dst = pend.pop(0)
                        rope_tail(raw, a, ct, dst)

                    return em

                qks = []
                for qk, w_sb, dst in ((0, wq_sb, qT_sb), (1, wk_sb, kT_sb)):
                    for ct in range(CT):
                        qks.append(mk_mm(ct, qk, w_sb, dst))
                ems.append(qks[0])
                ems.append(qks[1])
                ems.append(mk_tail())
                ems.append(qks[2])
                ems.append(mk_tail())
                ems.append(qks[3])
                ems.append(mk_tail())
                ems.append(lambda: v_group(0))
                ems.append(mk_tail())
                for j in range(1, KPQ):
                    ems.append(lambda j=j: v_group(j))
                return ems

            def emit_qkv(st):
                for em in qkv_groups(st):
                    em()

            attn_state = {}

            def emit_attn_main(t, fillers=(), start=0, reserve=0, inject=None):
                fillers = list(fillers)
                spread = fillers[: len(fillers) - reserve]
                reserved = fillers[len(fillers) - reserve :]
                qrot["pre"] = False
                qsl = slice(t * qt, (t + 1) * qt)
                ncc = KPQ * (t + 1)
                # AV/den psum banks: av01 pair-packs heads 0,1 [128, qt];
                # av2/av3 hold heads 2,3 as [0:64) AV + row 64 den, plus the
                # dens of heads 0,1 parked at row 96 by dedicated den matmuls.
                # Allocated lazily (first AV use) so the previous tile's
                # epilogue pbc tiles can be injected into this tile's weave
                # without inverting the av-bank tag order.
                pav = {}
                e_ts = {}
                scnt = {"i": 0}

                def emit_scores(c, hp):
                    j = c - KPQ * t
                    lo = max(0, j * 128)
                    pss = ps.tile(
                        [128, 2, qt], F32, tag=f"sc{'AB'[scnt['i'] % 2]}",
                        name=f"pss{t}_{c}_{hp}",
                    )
                    scnt["i"] += 1
                    for g in range(2):
                        bp = 64 * g
                        nc.tensor.matmul(
                            pss[:, g, lo:qt],
                            kT_sb[hp][bp : bp + dh, c * 128 : (c + 1) * 128],
                            qT_sb[hp][bp : bp + dh, t * qt + lo : (t + 1) * qt],
                            start=True,
                            stop=True,
                        )
                    e_t = expp.tile([128, 2, qt], BF16, tag="e", name="e")
                    nc.scalar.activation(
                        e_t[:, :, lo:qt], pss[:, :, lo:qt],
                        mybir.ActivationFunctionType.Exp, scale=scale,
                    )
                    if j >= 0:
                        # causal mask on the diagonal block: keep where
                        # query offset >= key offset
                        nc.gpsimd.affine_select(
                            out=e_t[:, :, lo : lo + 128],
                            in_=e_t[:, :, lo : lo + 128],
                            compare_op=mybir.AluOpType.is_ge,
                            fill=0.0,
                            base=0,
                            channel_multiplier=-1,
                            pattern=[[0, 2], [1, 128]],
                        )
                    e_ts[(c, hp)] = e_t

                def get_pav():
                    if "01" not in pav:
                        pav["01"] = ps.tile([128, qt], F32, tag="av01", name=f"pav{t}_01")
                        pav["2"] = ps.tile([97, qt], F32, tag="av2", name=f"pav{t}_2")
                        pav["3"] = ps.tile([97, qt], F32, tag="av3", name=f"pav{t}_3")
                    return pav

                def emit_av(c, hp):
                    lo = max(0, (c - KPQ * t) * 128)
                    e_t = e_ts.pop((c, hp))
                    p = get_pav()
                    st_fl = (c == 0)
                    sp_fl = (c == ncc - 1)
                    if hp == 0:
                        for g in range(2):
                            nc.tensor.matmul(
                                p["01"][64 * g : 64 * g + 64, lo:qt],
                                v_sb[c][:, g, 0:dh],
                                e_t[:, g, lo:qt],
                                start=st_fl, stop=sp_fl,
                                tile_position=(0, 64 * g),
                            )
                        # dens of heads 0,1 ride at row 96 of av2/av3
                        for g in range(2):
                            nc.tensor.matmul(
                                p[str(2 + g)][96:97, lo:qt],
                                v_sb[c][:, g, dh : dh + 1],
                                e_t[:, g, lo:qt],
                                start=st_fl, stop=sp_fl,
                                tile_position=(0, 96),
                            )
                    else:
                        for g in range(2):
                            nc.tensor.matmul(
                                p[str(2 + g)][0:65, lo:qt],
                                v_sb[c][:, 2 + g, :],
                                e_t[:, g, lo:qt],
                                start=st_fl, stop=sp_fl,
                            )

                done = 0
                nspread = len(spread)

                def advance(frac):
                    # weave filler groups: the PE keeps dense work that
                    # doesn't depend on ACT's exp stream
                    nonlocal done
                    if nspread:
                        want = min(nspread, int(frac * nspread))
                        while done < want:
                            spread[done]()
                            done += 1

                for c in range(ncc):
                    # interleave AV matmuls between the two serialized score
                    # pairs so the PE has work while ACT runs each exp
                    emit_scores(c, 0)
                    if c >= lag:
                        emit_av(c - lag, 0)
                    if c >= start:
                        advance((c - start + 0.5) / max(1, ncc - start))
                    emit_scores(c, 1)
                    if c >= lag:
                        emit_av(c - lag, 1)
                    if inject is not None and c == 2:
                        # previous tile's epilogue: its recip chain overlaps
                        # this tile's early chunks instead of blocking the PE
                        inject()
                        inject = None
                    if c >= start:
                        advance((c - start + 1.0) / max(1, ncc - start))
                for c in range(max(0, ncc - lag), ncc):
                    emit_av(c, 0)
                    emit_av(c, 1)
                advance(1.0)

                # softmax denominator path: aligned down-copies of the den
                # rows into rrA rows {0,32,64} (h=0..2) and rrB row 0 (h=3);
                # recip = exp(-ln(d)) on ACT
                p = get_pav()
                # on the last tile ACT's exp stream is done: split the gather
                # and u copies across DVE and ACT to halve the tail chain
                alt = nc.scalar.copy if t == NQT - 1 else nc.vector.tensor_copy
                nc.vector.tensor_copy(rrA_f[0:1, :], p["2"][96:97, :])
                alt(rrA_f[32:33, :], p["3"][96:97, :])
                nc.vector.tensor_copy(rrA_f[64:65, :], p["2"][64:65, :])
                alt(rrB_f, p["3"][64:65, :])
                lnA = bcp.tile([65, qt], F32, tag="lnA", name="lnA")
                lnB = bcp.tile([1, qt], F32, tag="lnB", name="lnB")
                nc.scalar.activation(lnA, rrA_f, mybir.ActivationFunctionType.Ln)
                nc.scalar.activation(lnB, rrB_f, mybir.ActivationFunctionType.Ln)
                rrA = bcp.tile([65, qt], BF16, tag="rrA", name="rrA")
                rrB = bcp.tile([1, qt], BF16, tag="rrB", name="rrB")
                with nc.allow_low_precision(reason="bf16 softmax recip"):
                    nc.scalar.activation(
                        rrA, lnA, mybir.ActivationFunctionType.Exp, scale=-1.0
                    )
                    nc.scalar.activation(
                        rrB, lnB, mybir.ActivationFunctionType.Exp, scale=-1.0
                    )
                attn_state[t] = (rrA, rrB)

                # unnormalized AV -> SBUF, per head (aligned/down copies)
                nc.vector.tensor_copy(u_sb[0][t], p["01"][0:64, :])
                alt(u_sb[1][t], p["01"][64:128, :])
                nc.vector.tensor_copy(u_sb[2][t], p["2"][0:64, :])
                alt(u_sb[3][t], p["3"][0:64, :])

                # reserved fillers keep the PE busy while the recip chain runs
                for f in reserved:
                    f()

            def pso_closures(t):
                ems = []
                for j in range(KPQ):
                    sb = t * KPQ + j

                    def em(sb=sb):
                        o_t = fo.tile([128, 2, 512], BF16, tag="ot", name="ot")
                        for nt in range(2):
                            pso = alloc_fill(512, f"pso{sb}_{nt}")
                            for kc in range(OC):
                                nc.tensor.matmul(
                                    pso,
                                    outT_sb[kc][:, sb * 128 : (sb + 1) * 128],
                                    wo_sb[:, kc, nt * 512 : (nt + 1) * 512],
                                    start=(kc == 0),
                                    stop=(kc == OC - 1),
                                )
                            # in the drain tail (pre-mode), ACT is idle: put
                            # the psum->sbuf copies there, freeing DVE for
                            # the epi multiplies
                            if qrot["pre"]:
                                nc.scalar.copy(o_t[:, nt, :], pso)
                            else:
                                nc.vector.tensor_copy(o_t[:, nt, :], pso)
                        nc.sync.dma_start(
                            outp[sb * 128 : (sb + 1) * 128, :],
                            o_t.rearrange("p a b -> p (a b)"),
                        )

                    ems.append(em)
                return ems

            def emit_epi_head(t, col_split=False, interleave=None):
                qsl = slice(t * qt, (t + 1) * qt)
                rrA, rrB = attn_state.pop(t)
                # broadcast recips: pbc01 pair-packs heads 0,1; pbc23 heads
                # 2,3 (head 3 via a ones-vector matmul into partitions 64+)
                pbc01 = ps.tile([128, qt], F32, tag="av01", name=f"pbc{t}_01")
                nc.tensor.matmul(pbc01, selc_sb[:, 0:128], rrA, start=True, stop=True)
                pbc23 = ps.tile([128, qt], F32, tag="av2", name=f"pbc{t}_23")
                nc.tensor.matmul(
                    pbc23[0:64, :], selc_sb[:, 128:192], rrA, start=True, stop=True
                )
                nc.tensor.matmul(
                    pbc23[64:128, :], ones1_sb, rrB, start=True, stop=True,
                    tile_position=(0, 64),
                )
                pbc = [pbc01[0:64, :], pbc01[64:128, :], pbc23[0:64, :], pbc23[64:128, :]]
                if not col_split:
                    for h in range(hl):
                        nc.vector.tensor_tensor(
                            outT_sb[h // 2][64 * (h % 2) : 64 * (h % 2) + 64, qsl],
                            u_sb[h][t],
                            pbc[h],
                            mybir.AluOpType.mult,
                        )
                    return
                # column-split: pipeline the epi multiply with the trailing
                # out-projection so the kernel tail drains block by block
                interleave = list(interleave or [])
                for j in range(KPQ):
                    cs = slice(j * 128, (j + 1) * 128)
                    gs = slice(t * qt + j * 128, t * qt + (j + 1) * 128)
                    for h in range(hl):
                        nc.vector.tensor_tensor(
                            outT_sb[h // 2][64 * (h % 2) : 64 * (h % 2) + 64, gs],
                            u_sb[h][t][:, cs],
                            pbc[h][:, cs],
                            mybir.AluOpType.mult,
                        )
                    if j < len(interleave):
                        interleave[j]()

            # fillers skewed toward the later (longer, ACT-bound) tiles;
            # each tile's epilogue is injected into the next tile's weave
            emit_qkv(0)
            emit_attn_main(0, qkv_groups(1), reserve=2)
            emit_attn_main(1, qkv_groups(2), reserve=2,
                           inject=lambda: emit_epi_head(0))
            emit_attn_main(2, qkv_groups(3) + pso_closures(0), reserve=3,
                           inject=lambda: emit_epi_head(1))
            emit_attn_main(3, pso_closures(1) + pso_closures(2), start=1, reserve=2,
                           inject=lambda: emit_epi_head(2))
            # drain tail: the score slots are free again — rotate the final
            # out-projection through three psum banks
            qrot["pre"] = True
            emit_epi_head(3, col_split=True, interleave=pso_closures(3))

    if cap_waits:
        _cap_matmul_waits(nc)
    return nc


_CAPPED_INSTS = {
    "InstMatmult",
    "InstTensorTensor",
    "InstTensorCopy",
    "InstActivation",
    "InstTensorScalarAffineSelect",
    "InstTensorScalar",
    "InstTensorReduce",
    "InstMemset",
    "InstReciprocal",
    "InstLdweights",
    "InstTensorTensorScan",
    "InstIota",
    "InstDMACopy",
    "InstDrain",
}


def _cap_matmul_waits(nc, max_keep=1):
    """Walrus codegen allows only one sync-wait per compute instruction
    (S3 struct wait slots).  Move excess waits onto NoOps inserted just
    before, on the same engine; engines execute in order so the semantics
    are identical."""
    nop_id = 0
    for f in nc.m.functions:
        for blk in f.blocks:
            insts = blk.instructions
            idx = 0
            while idx < len(insts):
                inst = insts[idx]
                if (
                    type(inst).__name__ in _CAPPED_INSTS
                    and inst.sync_info is not None
                    and len(inst.sync_info.on_wait or []) > max_keep
                ):
                    waits = list(inst.sync_info.on_wait)
                    extra, keep = waits[:-max_keep], waits[-max_keep:]
                    inst.sync_info = mybir.SyncInfo(
                        on_wait=keep, on_update=list(inst.sync_info.on_update or [])
                    )
                    for w in extra:
                        nop = mybir.InstNoOp(name=f"I-mmwait-nop-{nop_id}")
                        nop_id += 1
                        nop.engine = inst.engine
                        nop.sync_info = mybir.SyncInfo(on_wait=[w], on_update=[])
                        insts.insert(idx, nop)
                        idx += 1
                idx += 1


def _rope_tables(n, dh):
    """Host-side RoPE tables in transposed, 2-head-stacked, sign-folded form."""
    inv_freq = 1.0 / (10000.0 ** (np.arange(0, dh, 2, dtype=np.float32) / dh))
    t = np.arange(n, dtype=np.float32)
    freqs = np.outer(t, inv_freq).astype(np.float32)  # [n, dh/2]
    emb = np.concatenate([freqs, freqs], axis=-1)  # [n, dh]
    cos = np.cos(emb).astype(np.float32).T  # [dh, n]
    sin = np.sin(emb).astype(np.float32).T
    sin_signed = sin.copy()
    sin_signed[: dh // 2] *= -1.0
    cosT = np.ascontiguousarray(np.tile(cos, (128 // dh, 1)))
    sinT = np.ascontiguousarray(np.tile(sin_signed, (128 // dh, 1)))
    return cosT, sinT


_NC_CACHE = {}


def kernel(x, w_qkv, w_out):
    return run(x, w_qkv, w_out)[0]


def _bf16(a):
    import ml_dtypes

    return np.asarray(a, dtype=np.float32).astype(ml_dtypes.bfloat16)


def run(x, w_qkv, w_out, trace=False, build_kwargs=None):
    from concourse.bass_utils import run_bass_kernel_spmd

    x = np.asarray(x, dtype=np.float32)
    w_qkv = np.asarray(w_qkv, dtype=np.float32)
    w_out = np.asarray(w_out, dtype=np.float32)

    cosT, sinT = _rope_tables(N, DH)
    # selector for the recip-broadcast matmuls: cols 0:128 map rrA rows
    # {0,32} to heads 0,1 of pbc01; cols 128:192 map row 64 to pbc23[0:64]
    # (head 3 uses a ones vector against rrB)
    selm = np.zeros((65, 192), dtype=np.float32)
    selm[0, 0:64] = 1.0
    selm[32, 64:128] = 1.0
    selm[64, 128:192] = 1.0
    # rotate_half permutation: permM[r, p] = 1 iff r = swap(p)
    permM = np.zeros((128, 128), dtype=np.float32)
    for p in range(128):
        sw = p + 32 if (p % 64) < 32 else p - 32
        permM[sw, p] = 1.0
    in_maps = []
    for core in range(N_CORES):
        b = core // (N_CORES // B)
        g = core % (N_CORES // B)
        cs = slice(g * HL * DH, (g + 1) * HL * DH)
        # x[b].T is [d, n]; reblock into NQT contiguous [d, qt] column blocks
        xt = np.ascontiguousarray(x[b].T)  # [d, n]
        xt_blocks = np.concatenate(
            [xt[:, s * QT : (s + 1) * QT] for s in range(NQT)], axis=0
        )  # [NQT*d, qt]
        in_maps.append(
            {
                "xT": _bf16(xt_blocks),
                "wq": _bf16(w_qkv[:, cs]),
                "wk": _bf16(w_qkv[:, D:][:, cs]),
                "wv": _bf16(w_qkv[:, 2 * D :][:, cs]),
                "wo": _bf16(w_out[cs, :]),
                "cosT": _bf16(cosT),
                "sinT": _bf16(sinT),
                "selc": _bf16(selm),
                "perm": _bf16(permM),
            }
        )

    key = repr(sorted((build_kwargs or {}).items()))
    if key not in _NC_CACHE:
        _NC_CACHE[key] = build_attention_nc(**(build_kwargs or {}))
    nc = _NC_CACHE[key]

    res = run_bass_kernel_spmd(
        nc, in_maps, core_ids=list(range(N_CORES)), trace=trace
    )
    out = np.zeros((B, N, D), dtype=np.float32)
    for core in range(N_CORES):
        out[core // (N_CORES // B)] += np.asarray(
            res.results[core]["out_partial"], dtype=np.float32
        )
    return out, res


if __name__ == "__main__":
    rng = np.random.default_rng(0)
    x = rng.standard_normal((B, N, D), dtype=np.float32)
    w_qkv = rng.standard_normal((D, 3 * D), dtype=np.float32) * D**-0.5
    w_out = rng.standard_normal((D, D), dtype=np.float32) * D**-0.5
    out = kernel(x, w_qkv, w_out)
    print("out", out.shape, out.dtype, float(np.abs(out).max()))
